# revision 15
# baseline (speedup 1.0000x reference)
"""Trainium2 Bass kernel for nn_Block (dense transformer block).

Sharding: pure data-parallel over batch — 16 batch elements, 2 per core,
no collectives. Each core runs the full block on its 2 batch elements.

Design (per core, per batch element):
  - LN1 in token-major layout (free-dim stats via bn_stats/bn_aggr;
    rstd = exp(-0.5*log(var+eps)) so only the Exp/Log ACT table set is used)
  - h transposed to feature-major hT [D, T] via PE transposes
  - Q/K computed feature-major (heads packed in pairs at partition offsets
    {0, 64}); V computed token-major with a per-head ones column appended
  - scores computed DIRECTLY transposed: sT[s, t] = kT.T @ qT, so softmax
    exp needs only a global constant bias (-C), no per-row max
  - causal masking: matmul N-range restriction (left of diagonal) + a bf16
    identity@mask matmul adding -1e9 on the diagonal blocks
  - AV feature-major: attnT[e, t] = v_ext.T @ expT; the ones column yields
    the softmax denominators l as rows 56/120 of the psum
  - per-head 1/l: l rows gathered via tiny SBUF->SBUF DMAs, recip via ACT
    Log+Exp, broadcast back to 128 partitions via an indicator matmul,
    applied as one in-place tensor_tensor multiply
  - proj token-major (lhsT = attnT slices; heads at offsets {0,64} packed
    into K via tile_position row groups), +residual +b_proj
  - LN2, fc1 feature-major + relu (+b1), fc2 token-major (+b2) + residual

All matmuls run in float32r (fp32 rounded to 11-bit mantissa, RNE —
verified bit-exact against HW), which streams at 1 cycle/row like bf16.
LN gains/biases are folded into adjacent matmul weights host-side.
"""
import sys, os
sys.path.insert(0, "/opt/trn_rl_repo")
import math
import numpy as np

import concourse.bass as bass
import concourse.bacc as bacc
import concourse.tile as tile
from concourse import mybir
from concourse.bass_utils import run_bass_kernel_spmd
from ml_dtypes import bfloat16

P = 128
B, T, D, H = 16, 1024, 448, 8
HS = D // H            # 56
DH = 4 * D             # 1792
NT = T // P            # 8 t-tiles per batch element
NK = 4                 # d k-tiles (448 padded to 512)
NM = DH // P           # 14
NCORE = 8
BPC = B // NCORE       # 2 batch elements per core
EPS = 1e-5
SCALE = 1.0 / math.sqrt(D)
C = 4.0                # global exp bias; max |scaled score| measured 0.575
NEG = -1.0e9
CH = 512               # attention t-chunk
F2 = 256               # mlp t-chunk

f32 = mybir.dt.float32
f32r = mybir.dt.float32r
bf16 = mybir.dt.bfloat16
AF = mybir.ActivationFunctionType
ALU = mybir.AluOpType


def _r12(a):
    """f32r rounding: RNE to 11-bit mantissa (bit-exact vs HW)."""
    b = np.ascontiguousarray(np.asarray(a, np.float32)).view(np.uint32).astype(np.uint64)
    r = (b + 0x7FF + ((b >> 12) & 1)) & np.uint64(0xFFFFF000)
    return r.astype(np.uint32).view(np.float32)


def _prep(inputs):
    """Host-side weight packing/folding. Returns dict of shared arrays."""
    wq, wk, wv = inputs["wq"], inputs["wk"], inputs["wv"]
    g1, bb1 = inputs["ln1_g"], inputs["ln1_b"]
    g2, bb2 = inputs["ln2_g"], inputs["ln2_b"]
    w_proj, w1, w2 = inputs["w_proj"], inputs["w1"], inputs["w2"]
    b1 = inputs["b1"]

    wq_e = (wq * g1[None, None, :]).astype(np.float32)   # [H, HS, D]
    wk_e = (wk * g1[None, None, :]).astype(np.float32)
    wv_e = (wv * g1[None, None, :]).astype(np.float32)
    qbias = np.einsum('hed,d->he', wq_e, bb1).astype(np.float32)
    kbias = np.einsum('hed,d->he', wk_e, bb1).astype(np.float32)
    vbias = np.einsum('hed,d->he', wv_e, bb1).astype(np.float32)

    # Q/K lhsT: [p, qk, pair, ktile, col] — head 2p at cols 0-55, 2p+1 at 64-119
    wqk = np.zeros((2, 4, NK, P, P), np.float32)
    for qk, W in enumerate((wq_e, wk_e)):
        for pr in range(4):
            for kk in range(NK):
                lo, hi = 128 * kk, min(128 * kk + 128, D)
                n = hi - lo
                wqk[qk, pr, kk, :n, 0:56] = W[2 * pr][:, lo:hi].T
                wqk[qk, pr, kk, :n, 64:120] = W[2 * pr + 1][:, lo:hi].T
    wqk_h = _r12(wqk.transpose(3, 0, 1, 2, 4).copy())     # [128, 2, 4, 4, 128]

    qkb = np.zeros((P, 2, 4), np.float32)                 # bias cols for q/k evict
    for qk, bias in enumerate((qbias, kbias)):
        for pr in range(4):
            qkb[0:56, qk, pr] = bias[2 * pr]
            qkb[64:120, qk, pr] = bias[2 * pr + 1]

    # V rhs: [ktile, row, outfeat 448] head-major; vbias row in [8,56] layout
    wvt = np.zeros((NK, P, D), np.float32)
    for kk in range(NK):
        lo, hi = 128 * kk, min(128 * kk + 128, D)
        for h in range(H):
            wvt[kk, :hi - lo, 56 * h:56 * h + 56] = wv_e[h][:, lo:hi].T
    wvt_h = _r12(wvt.transpose(1, 0, 2).copy())           # [128, 4, 448]
    vb_h = vbias.reshape(D).astype(np.float32)            # [448] head-major

    # proj rhs: rows packed to match attnT (head 2p at 0-55, 2p+1 at 64-119)
    wpT = w_proj.T.astype(np.float32)                     # [in, out]
    wpt = np.zeros((4, P, D), np.float32)
    for pr in range(4):
        wpt[pr, 0:56, :] = wpT[56 * (2 * pr):56 * (2 * pr) + 56, :]
        wpt[pr, 64:120, :] = wpT[56 * (2 * pr + 1):56 * (2 * pr + 1) + 56, :]
    wpt_h = _r12(wpt.transpose(1, 0, 2).copy())           # [128, 4, 448]

    # fc1 lhsT tiles [ktile, mtile, row, col]; fc2 rhs tiles [ktile, row, 448]
    w1_e = (w1 * g2[None, :]).astype(np.float32)          # [DH, D]
    b1_e = (b1 + w1_e @ bb2).astype(np.float32)
    w1T = np.zeros((512, DH), np.float32)
    w1T[:D] = w1_e.T
    w1t_h = _r12(w1T.reshape(NK, P, NM, P).transpose(1, 0, 2, 3).copy())  # [128,4,14,128]
    b1c_h = b1_e.reshape(NM, P).T.copy()                  # [128, 14]
    w2t_h = _r12(w2.T.reshape(NM, P, D).transpose(1, 0, 2).copy())        # [128,14,448]

    # indicator for per-head recip broadcast: [8, pair, 128]
    ind = np.zeros((H, 4, P), np.float32)
    for pr in range(4):
        ind[2 * pr, pr, 0:56] = 1.0
        ind[2 * pr + 1, pr, 64:120] = 1.0

    # bf16 identity + causal mask (in [s_local, t_local]: mask s>t)
    ident_b = np.eye(P, dtype=np.float32).astype(bfloat16)
    tri = np.where(np.arange(P)[:, None] > np.arange(P)[None, :],
                   np.float32(NEG), np.float32(0.0))
    maskb = tri.astype(bfloat16)
    maskw = np.concatenate(
        [np.full((P, P), np.float32(NEG)), tri], axis=1).astype(bfloat16)

    return {
        "wqk": wqk_h, "qkb": qkb, "wvt": wvt_h, "vb": vb_h,
        "wpt": wpt_h, "bproj": inputs["b_proj"].astype(np.float32),
        "w1t": w1t_h, "b1c": b1c_h, "w2t": w2t_h,
        "b2": inputs["b2"].astype(np.float32),
        "ind": _r12(ind), "identb": ident_b, "maskb": maskb, "maskw": maskw,
        "idr": _r12(np.eye(P, dtype=np.float32)),
        "ones1": np.ones(H, np.float32).astype(bfloat16),
        "zeros1": _r12(np.zeros(T, np.float32)),
    }


def _pb(ap, n, extra=None):
    """Prepend a step-0 partition-broadcast dim (+optional middle dims)."""
    dims = [[0, n]] + (extra or []) + list(ap.ap)
    return bass.AP(tensor=ap.tensor, offset=ap.offset, ap=dims)


def _build():
    KPHASE = int(os.environ.get("KPHASE", "5"))
    nc = bacc.Bacc(None, target_bir_lowering=False, debug=False)

    x_d = nc.dram_tensor("x", [BPC, T, D], f32, kind="ExternalInput")
    wqk_d = nc.dram_tensor("wqk", [P, 2, 4, NK, P], f32r, kind="ExternalInput")
    qkb_d = nc.dram_tensor("qkb", [P, 2, 4], f32, kind="ExternalInput")
    wvt_d = nc.dram_tensor("wvt", [P, NK, D], f32r, kind="ExternalInput")
    vb_d = nc.dram_tensor("vb", [D], f32, kind="ExternalInput")
    wpt_d = nc.dram_tensor("wpt", [P, 4, D], f32r, kind="ExternalInput")
    bproj_d = nc.dram_tensor("bproj", [D], f32, kind="ExternalInput")
    w1t_d = nc.dram_tensor("w1t", [P, NK, NM, P], f32r, kind="ExternalInput")
    b1c_d = nc.dram_tensor("b1c", [P, NM], f32, kind="ExternalInput")
    w2t_d = nc.dram_tensor("w2t", [P, NM, D], f32r, kind="ExternalInput")
    b2_d = nc.dram_tensor("b2", [D], f32, kind="ExternalInput")
    ind_d = nc.dram_tensor("ind", [H, 4, P], f32r, kind="ExternalInput")
    identb_d = nc.dram_tensor("identb", [P, P], bf16, kind="ExternalInput")
    maskb_d = nc.dram_tensor("maskb", [P, P], bf16, kind="ExternalInput")
    maskw_d = nc.dram_tensor("maskw", [P, 2 * P], bf16, kind="ExternalInput")
    idr_d = nc.dram_tensor("idr", [P, P], f32r, kind="ExternalInput")
    ones_d = nc.dram_tensor("ones1", [H], bf16, kind="ExternalInput")
    zeros_d = nc.dram_tensor("zeros1", [T], f32r, kind="ExternalInput")
    out_d = nc.dram_tensor("out", [BPC, T, D], f32, kind="ExternalOutput")

    with tile.TileContext(nc) as tc:
        import contextlib
        ctx = contextlib.ExitStack()
        with ctx:
            const = ctx.enter_context(tc.tile_pool(name="const", bufs=1))
            pool1 = ctx.enter_context(tc.tile_pool(name="pool1", bufs=1))
            pool2 = ctx.enter_context(tc.tile_pool(name="pool2", bufs=2))
            pool3 = ctx.enter_context(tc.tile_pool(name="pool3", bufs=3))
            ps_sc = ctx.enter_context(tc.tile_pool(name="ps_sc", bufs=2, space="PSUM"))
            ps_av = ctx.enter_context(tc.tile_pool(name="ps_av", bufs=2, space="PSUM"))
            ps_ms = ctx.enter_context(tc.tile_pool(name="ps_ms", bufs=3, space="PSUM"))
            ps_bc = ctx.enter_context(tc.tile_pool(name="ps_bc", bufs=1, space="PSUM"))

            # ---- resident constants ----
            qkb_t = const.tile([P, 2, 4], f32)
            nc.sync.dma_start(out=qkb_t[:, :, :], in_=qkb_d[:, :, :])
            wvt_t = const.tile([P, NK, D], f32r)
            nc.sync.dma_start(out=wvt_t[:, :, :], in_=wvt_d[:, :, :])
            vb_t = const.tile([P, D], f32)
            nc.gpsimd.dma_start(out=vb_t[:, :], in_=_pb(vb_d[:], P))
            wpt_t = const.tile([P, 4, D], f32r)
            nc.sync.dma_start(out=wpt_t[:, :, :], in_=wpt_d[:, :, :])
            bproj_t = const.tile([P, D], f32)
            nc.gpsimd.dma_start(out=bproj_t[:, :], in_=_pb(bproj_d[:], P))
            b1c_t = const.tile([P, NM], f32)
            nc.sync.dma_start(out=b1c_t[:, :], in_=b1c_d[:, :])
            b2_t = const.tile([P, D], f32)
            nc.gpsimd.dma_start(out=b2_t[:, :], in_=_pb(b2_d[:], P))
            ind_t = const.tile([H, 4, P], f32r)
            nc.sync.dma_start(out=ind_t[:, :, :], in_=ind_d[:, :, :])
            identb_t = const.tile([P, P], bf16)
            nc.sync.dma_start(out=identb_t[:, :], in_=identb_d[:, :])
            maskb_t = const.tile([P, P], bf16)
            nc.sync.dma_start(out=maskb_t[:, :], in_=maskb_d[:, :])
            maskw_t = const.tile([P, 2 * P], bf16)
            nc.sync.dma_start(out=maskw_t[:, :], in_=maskw_d[:, :])
            idr_t = const.tile([P, P], f32r)
            nc.sync.dma_start(out=idr_t[:, :], in_=idr_d[:, :])
            eps_t = const.tile([P, 1], f32)
            nc.vector.memset(eps_t[:, :], EPS)
            negc_t = const.tile([P, 1], f32)
            nc.vector.memset(negc_t[:, :], -C)

            def layernorm_to_featT(src_t, tag_prefix):
                """src_t [128, NT, 448] fp32 -> featT [128, NK, 1024] f32r."""
                mv_t = pool3.tile([P, NT, 2], f32, tag="mv")
                for i in range(NT):
                    st = pool3.tile([P, 6], f32, tag="stats")
                    nc.vector.bn_stats(out=st[:, :], in_=src_t[:, i, :])
                    nc.vector.bn_aggr(out=mv_t[:, i, :], in_=st[:, :])
                lg_t = pool3.tile([P, NT], f32, tag="lg")
                nc.scalar.activation(lg_t[:, :], mv_t[:, :, 1], AF.Ln, bias=eps_t[:, 0:1])
                rstd_t = pool3.tile([P, NT], f32, tag="rstd")
                nc.scalar.activation(rstd_t[:, :], lg_t[:, :], AF.Exp, scale=-0.5)

                ft = pool1.tile([P, NK, T], f32r, tag="featT")
                nc.gpsimd.dma_start(
                    out=ft[64:128, 3, :],
                    in_=_pb(zeros_d[:], 64))
                for g in range(2):        # groups of 4 t-tiles
                    for kk in range(NK):
                        w = 128 if kk < 3 else 64
                        pt = ps_ms.tile([P, CH], f32r, tag="ps_ms")
                        for ii in range(4):
                            i = 4 * g + ii
                            h_t = pool3.tile([P, P], f32r, tag="h")
                            nc.vector.tensor_scalar(
                                out=h_t[:, 0:w],
                                in0=src_t[:, i, 128 * kk:128 * kk + w],
                                scalar1=mv_t[:, i, 0:1],
                                scalar2=rstd_t[:, i:i + 1],
                                op0=ALU.subtract, op1=ALU.mult)
                            nc.tensor.transpose(
                                pt[0:w, 128 * ii:128 * ii + 128],
                                h_t[:, 0:w], idr_t[:, :])
                        nc.vector.tensor_copy(
                            ft[0:w, kk, CH * g:CH * g + CH], pt[0:w, :])
                return ft

            for b in range(BPC):
                # ---- load x (also the residual base tile) ----
                xb_t = pool2.tile([P, NT, D], f32, tag="resid")
                nc.sync.dma_start(
                    out=xb_t[:, :, :],
                    in_=x_d[b].rearrange("(n p) d -> p n d", p=P))

                # ---- LN1 + transpose ----
                hT_t = layernorm_to_featT(xb_t, "ln1")

                # xb := x + b_proj (in place; after LN1 reads)
                for i in range(NT):
                    nc.vector.tensor_add(xb_t[:, i, :], xb_t[:, i, :], bproj_t[:, :])

                if KPHASE < 2:
                    continue
                # ---- QKV ----
                wqk_t = pool1.tile([P, 2, 4, NK, P], f32r, tag="shA")
                nc.sync.dma_start(out=wqk_t[:, :, :, :, :], in_=wqk_d[:, :, :, :, :])
                qT_t = pool1.tile([P, 4, T], f32r, tag="shQ")
                kT_t = pool1.tile([P, 4, T], f32r, tag="shK")
                for pr in range(4):
                    for c in range(2):
                        pq = ps_ms.tile([P, CH], f32, tag="ps_ms")
                        for kk in range(NK):
                            nc.tensor.matmul(
                                pq[:, :], wqk_t[:, 0, pr, kk, :],
                                hT_t[:, kk, CH * c:CH * c + CH],
                                start=(kk == 0), stop=(kk == NK - 1))
                        nc.scalar.activation(
                            qT_t[:, pr, CH * c:CH * c + CH], pq[:, :],
                            AF.Identity, bias=qkb_t[:, 0, pr:pr + 1])
                        pk = ps_ms.tile([P, CH], f32, tag="ps_ms")
                        for kk in range(NK):
                            nc.tensor.matmul(
                                pk[:, :], wqk_t[:, 1, pr, kk, :],
                                hT_t[:, kk, CH * c:CH * c + CH],
                                start=(kk == 0), stop=(kk == NK - 1))
                        nc.vector.tensor_scalar(
                            out=kT_t[:, pr, CH * c:CH * c + CH], in0=pk[:, :],
                            scalar1=qkb_t[:, 1, pr:pr + 1], scalar2=None,
                            op0=ALU.add)

                # ---- V (token-major, with ones column) ----
                vext_t = pool1.tile([P, NT, H, 57], bf16, tag="vext")
                nc.gpsimd.dma_start(
                    out=vext_t[:, :, :, 56:57],
                    in_=bass.AP(tensor=ones_d[:].tensor, offset=0,
                                ap=[[0, P], [0, NT * H], [1, 1]]))
                for i in range(NT):
                    pv = ps_ms.tile([P, CH], f32, tag="ps_ms")
                    for kk in range(NK):
                        nc.tensor.matmul(
                            pv[:, 0:D], hT_t[:, kk, 128 * i:128 * i + 128],
                            wvt_t[:, kk, :],
                            start=(kk == 0), stop=(kk == NK - 1))
                    nc.vector.tensor_add(
                        vext_t[:, i, :, 0:56],
                        pv[:, 0:D].rearrange("p (h e) -> p h e", h=H),
                        vb_t[:, :].rearrange("p (h e) -> p h e", h=H))

                if KPHASE < 3:
                    continue
                # ---- attention ----
                attnT_t = pool1.tile([P, 4, T], f32r, tag="shA")
                l_t = pool1.tile([H, T], f32r, tag="l")
                for pr in range(4):
                    for c in range(2):
                        pav = ps_av.tile([P, CH], f32, tag="ps_av")
                        for half in range(2):
                            h = 2 * pr + half
                            poff = 64 * half
                            jmax = 4 * c + 3
                            e_t = pool2.tile([P, 8, CH], bf16, tag="shE")
                            for j in range(jmax + 1):
                                co = max(0, 128 * (j - 4 * c))
                                if co == 384:
                                    co = 256   # keep matmul N >= 256 for f32r speed
                                pss = ps_sc.tile([P, CH], f32, tag="ps_sc")
                                diag = j >= 4 * c
                                nc.tensor.matmul(
                                    pss[:, co:CH],
                                    kT_t[poff:poff + 56, pr, 128 * j:128 * j + 128],
                                    qT_t[poff:poff + 56, pr, CH * c + co:CH * c + CH],
                                    start=True, stop=not diag)
                                if diag:
                                    dco = 128 * (j - 4 * c)
                                    if dco == 384:
                                        nc.tensor.matmul(
                                            pss[:, 256:512],
                                            identb_t[:, :], maskw_t[:, :],
                                            start=False, stop=True)
                                    else:
                                        nc.tensor.matmul(
                                            pss[:, dco:dco + 128],
                                            identb_t[:, :], maskb_t[:, :],
                                            start=False, stop=True)
                                nc.scalar.activation(
                                    e_t[:, j, co:CH], pss[:, co:CH],
                                    AF.Exp, scale=SCALE, bias=negc_t[:, 0:1])
                            for j in range(jmax + 1):
                                co = max(0, 128 * (j - 4 * c))
                                if co == 384:
                                    co = 256
                                nc.tensor.matmul(
                                    pav[poff:poff + 57, co:CH],
                                    vext_t[:, j, h, :], e_t[:, j, co:CH],
                                    tile_position=(0, poff),
                                    start=(j == 0), stop=(j == jmax))
                        nc.scalar.copy(
                            attnT_t[0:57, pr, CH * c:CH * c + CH], pav[0:57, :])
                        nc.vector.tensor_copy(
                            attnT_t[64:121, pr, CH * c:CH * c + CH], pav[64:121, :])
                        for half in range(2):
                            nc.sync.dma_start(
                                out=l_t[2 * pr + half:2 * pr + half + 1,
                                        CH * c:CH * c + CH],
                                in_=attnT_t[56 + 64 * half:57 + 64 * half, pr,
                                            CH * c:CH * c + CH])

                # recip of softmax denominators: 1/l = exp(-log(l))
                nc.scalar.activation(l_t[:, :], l_t[:, :], AF.Ln)
                recip_t = pool1.tile([H, T], f32r, tag="recip")
                nc.scalar.activation(recip_t[:, :], l_t[:, :], AF.Exp, scale=-1.0)

                for pr in range(4):
                    for c in range(2):
                        pb = ps_bc.tile([P, CH], f32, tag="ps_bc")
                        nc.tensor.matmul(
                            pb[:, :], ind_t[:, pr, :],
                            recip_t[:, CH * c:CH * c + CH],
                            start=True, stop=True)
                        nc.vector.tensor_mul(
                            attnT_t[0:57, pr, CH * c:CH * c + CH],
                            attnT_t[0:57, pr, CH * c:CH * c + CH], pb[0:57, :])
                        nc.vector.tensor_mul(
                            attnT_t[64:121, pr, CH * c:CH * c + CH],
                            attnT_t[64:121, pr, CH * c:CH * c + CH], pb[64:121, :])

                if KPHASE < 4:
                    continue
                # ---- proj + residual ----
                x2_t = pool2.tile([P, NT, D], f32, tag="resid")
                for i in range(NT):
                    ppe = ps_ms.tile([P, CH], f32, tag="ps_ms")
                    ppo = ps_ms.tile([P, CH], f32, tag="ps_ms")
                    for half in range(2):
                        poff = 64 * half
                        pp = ppe if half == 0 else ppo
                        for pr in range(4):
                            nc.tensor.matmul(
                                pp[:, 0:D],
                                attnT_t[poff:poff + 56, pr, 128 * i:128 * i + 128],
                                wpt_t[poff:poff + 56, pr, :],
                                start=(pr == 0), stop=(pr == 3))
                    t1_t = pool3.tile([P, D], f32, tag="projtmp")
                    nc.vector.tensor_add(t1_t[:, :], ppe[:, 0:D], xb_t[:, i, :])
                    nc.vector.tensor_add(x2_t[:, i, :], ppo[:, 0:D], t1_t[:, :])

                # ---- LN2 + transpose ----
                h2T_t = layernorm_to_featT(x2_t, "ln2")

                # x2 := x2 + b2 (in place; after LN2 reads)
                for i in range(NT):
                    nc.vector.tensor_add(x2_t[:, i, :], x2_t[:, i, :], b2_t[:, :])

                if KPHASE < 5:
                    continue
                # ---- MLP ----
                w1t_t = pool1.tile([P, NK, NM, P], f32r, tag="shQ")
                nc.sync.dma_start(out=w1t_t[:, :, :, :], in_=w1t_d[:, :, :, :])
                w2t_t = pool1.tile([P, NM, D], f32r, tag="shK")
                nc.sync.dma_start(out=w2t_t[:, :, :], in_=w2t_d[:, :, :])

                for c2 in range(4):          # 256-wide t-chunks
                    rel_t = pool2.tile([P, NM, F2], f32r, tag="shE")
                    for m in range(NM):
                        pf = ps_ms.tile([P, CH], f32, tag="ps_ms")
                        for kk in range(NK):
                            nc.tensor.matmul(
                                pf[:, 0:F2], w1t_t[:, kk, m, :],
                                h2T_t[:, kk, F2 * c2:F2 * c2 + F2],
                                start=(kk == 0), stop=(kk == NK - 1))
                        if m % 2 == 0:
                            nc.scalar.activation(
                                rel_t[:, m, :], pf[:, 0:F2],
                                AF.Relu, bias=b1c_t[:, m:m + 1])
                        else:
                            nc.vector.tensor_scalar(
                                out=rel_t[:, m, :], in0=pf[:, 0:F2],
                                scalar1=b1c_t[:, m:m + 1], scalar2=0.0,
                                op0=ALU.add, op1=ALU.max)
                    for ii in range(2):
                        i = 2 * c2 + ii
                        po = ps_ms.tile([P, CH], f32, tag="ps_ms")
                        for k14 in range(NM):
                            nc.tensor.matmul(
                                po[:, 0:D],
                                rel_t[:, k14, 128 * ii:128 * ii + 128],
                                w2t_t[:, k14, :],
                                start=(k14 == 0), stop=(k14 == NM - 1))
                        o_t = pool3.tile([P, D], f32, tag="outt")
                        nc.vector.tensor_add(o_t[:, :], po[:, 0:D], x2_t[:, i, :])
                        nc.sync.dma_start(
                            out=out_d[b, 128 * i:128 * i + 128, :], in_=o_t[:, :])

    nc.finalize()
    return nc


_CACHE = {}


def run(inputs, trace=False):
    if "nc" not in _CACHE:
        _CACHE["nc"] = _build()
    nc = _CACHE["nc"]
    host = _prep({k: np.asarray(v) for k, v in inputs.items()})
    x = np.asarray(inputs["x"], np.float32)
    in_maps = []
    for cidx in range(NCORE):
        m = dict(host)
        m["x"] = np.ascontiguousarray(x[BPC * cidx:BPC * cidx + BPC])
        in_maps.append(m)
    r = run_bass_kernel_spmd(nc, in_maps, list(range(NCORE)), trace=trace)
    out = np.concatenate([r.results[cidx]["out"] for cidx in range(NCORE)], axis=0)
    return out, r


def kernel(**inputs):
    out, _ = run(inputs, trace=False)
    return out


# revision 19
# speedup vs baseline: 1.0250x; 1.0250x over previous
"""Trainium2 Bass kernel for nn_Block (dense transformer block).

Sharding: pure data-parallel over batch — 16 batch elements, 2 per core,
no collectives. Each core runs the full block on its 2 batch elements.

Design (per core, per batch element):
  - LN1 in token-major layout (free-dim stats via bn_stats/bn_aggr;
    rstd = exp(-0.5*log(var+eps)) so only the Exp/Log ACT table set is used)
  - h transposed to feature-major hT [D, T] via PE transposes
  - Q/K computed feature-major (heads packed in pairs at partition offsets
    {0, 64}); V computed token-major with a per-head ones column appended
  - scores computed DIRECTLY transposed: sT[s, t] = kT.T @ qT, so softmax
    exp needs only a global constant bias (-C), no per-row max
  - causal masking: matmul N-range restriction (left of diagonal) + a bf16
    identity@mask matmul adding -1e9 on the diagonal blocks
  - AV feature-major: attnT[e, t] = v_ext.T @ expT; the ones column yields
    the softmax denominators l as rows 56/120 of the psum
  - per-head 1/l: l rows gathered via tiny SBUF->SBUF DMAs, recip via ACT
    Log+Exp, broadcast back to 128 partitions via an indicator matmul,
    applied as one in-place tensor_tensor multiply
  - proj token-major (lhsT = attnT slices; heads at offsets {0,64} packed
    into K via tile_position row groups), +residual +b_proj
  - LN2, fc1 feature-major + relu (+b1), fc2 token-major (+b2) + residual

All matmuls run in float32r (fp32 rounded to 11-bit mantissa, RNE —
verified bit-exact against HW), which streams at 1 cycle/row like bf16.
LN gains/biases are folded into adjacent matmul weights host-side.
"""
import sys, os
sys.path.insert(0, "/opt/trn_rl_repo")
import math
import numpy as np

import concourse.bass as bass
import concourse.bacc as bacc
import concourse.tile as tile
from concourse import mybir
from concourse.bass_utils import run_bass_kernel_spmd
from ml_dtypes import bfloat16

P = 128
B, T, D, H = 16, 1024, 448, 8
HS = D // H            # 56
DH = 4 * D             # 1792
NT = T // P            # 8 t-tiles per batch element
NK = 4                 # d k-tiles (448 padded to 512)
NM = DH // P           # 14
NCORE = 8
BPC = B // NCORE       # 2 batch elements per core
EPS = 1e-5
SCALE = 1.0 / math.sqrt(D)
C = 4.0                # global exp bias; max |scaled score| measured 0.575
NEG = -1.0e9
CH = 512               # attention t-chunk
F2 = 256               # mlp t-chunk

f32 = mybir.dt.float32
f32r = mybir.dt.float32r
bf16 = mybir.dt.bfloat16
AF = mybir.ActivationFunctionType
ALU = mybir.AluOpType


def _r12(a):
    """f32r rounding: RNE to 11-bit mantissa (bit-exact vs HW)."""
    b = np.ascontiguousarray(np.asarray(a, np.float32)).view(np.uint32).astype(np.uint64)
    r = (b + 0x7FF + ((b >> 12) & 1)) & np.uint64(0xFFFFF000)
    return r.astype(np.uint32).view(np.float32)


def _prep(inputs):
    """Host-side weight packing/folding. Returns dict of shared arrays."""
    wq, wk, wv = inputs["wq"], inputs["wk"], inputs["wv"]
    g1, bb1 = inputs["ln1_g"], inputs["ln1_b"]
    g2, bb2 = inputs["ln2_g"], inputs["ln2_b"]
    w_proj, w1, w2 = inputs["w_proj"], inputs["w1"], inputs["w2"]
    b1 = inputs["b1"]

    wq_e = (wq * g1[None, None, :]).astype(np.float32)   # [H, HS, D]
    wk_e = (wk * g1[None, None, :]).astype(np.float32)
    wv_e = (wv * g1[None, None, :]).astype(np.float32)
    qbias = np.einsum('hed,d->he', wq_e, bb1).astype(np.float32)
    kbias = np.einsum('hed,d->he', wk_e, bb1).astype(np.float32)
    vbias = np.einsum('hed,d->he', wv_e, bb1).astype(np.float32)

    # Q/K lhsT: [p, qk, pair, ktile, col] — head 2p at cols 0-55, 2p+1 at 64-119
    wqk = np.zeros((2, 4, NK, P, P), np.float32)
    for qk, W in enumerate((wq_e, wk_e)):
        for pr in range(4):
            for kk in range(NK):
                lo, hi = 128 * kk, min(128 * kk + 128, D)
                n = hi - lo
                wqk[qk, pr, kk, :n, 0:56] = W[2 * pr][:, lo:hi].T
                wqk[qk, pr, kk, :n, 64:120] = W[2 * pr + 1][:, lo:hi].T
    wqk_h = _r12(wqk.transpose(3, 0, 1, 2, 4).copy())     # [128, 2, 4, 4, 128]

    qkb = np.zeros((P, 2, 4), np.float32)                 # bias cols for q/k evict
    for qk, bias in enumerate((qbias, kbias)):
        for pr in range(4):
            qkb[0:56, qk, pr] = bias[2 * pr]
            qkb[64:120, qk, pr] = bias[2 * pr + 1]

    # V rhs: [ktile, row, outfeat 448] head-major; vbias row in [8,56] layout
    wvt = np.zeros((NK, P, D), np.float32)
    for kk in range(NK):
        lo, hi = 128 * kk, min(128 * kk + 128, D)
        for h in range(H):
            wvt[kk, :hi - lo, 56 * h:56 * h + 56] = wv_e[h][:, lo:hi].T
    wvt_h = _r12(wvt.transpose(1, 0, 2).copy())           # [128, 4, 448]
    vb_h = vbias.reshape(D).astype(np.float32)            # [448] head-major

    # proj rhs: rows packed to match attnT (head 2p at 0-55, 2p+1 at 64-119)
    wpT = w_proj.T.astype(np.float32)                     # [in, out]
    wpt = np.zeros((4, P, D), np.float32)
    for pr in range(4):
        wpt[pr, 0:56, :] = wpT[56 * (2 * pr):56 * (2 * pr) + 56, :]
        wpt[pr, 64:120, :] = wpT[56 * (2 * pr + 1):56 * (2 * pr + 1) + 56, :]
    wpt_h = _r12(wpt.transpose(1, 0, 2).copy())           # [128, 4, 448]

    # fc1 lhsT tiles [ktile, mtile, row, col]; fc2 rhs tiles [ktile, row, 448]
    w1_e = (w1 * g2[None, :]).astype(np.float32)          # [DH, D]
    b1_e = (b1 + w1_e @ bb2).astype(np.float32)
    w1T = np.zeros((512, DH), np.float32)
    w1T[:D] = w1_e.T
    w1t_h = _r12(w1T.reshape(NK, P, NM, P).transpose(1, 0, 2, 3).copy())  # [128,4,14,128]
    b1c_h = b1_e.reshape(NM, P).T.copy()                  # [128, 14]
    w2t_h = _r12(w2.T.reshape(NM, P, D).transpose(1, 0, 2).copy())        # [128,14,448]

    # indicator for per-head recip broadcast: [8, pair, 128]
    ind = np.zeros((H, 4, P), np.float32)
    for pr in range(4):
        ind[2 * pr, pr, 0:56] = 1.0
        ind[2 * pr + 1, pr, 64:120] = 1.0

    # bf16 identity + causal mask (in [s_local, t_local]: mask s>t)
    ident_b = np.eye(P, dtype=np.float32).astype(bfloat16)
    tri = np.where(np.arange(P)[:, None] > np.arange(P)[None, :],
                   np.float32(NEG), np.float32(0.0))
    maskb = tri.astype(bfloat16)
    maskw = np.concatenate(
        [np.full((P, P), np.float32(NEG)), tri], axis=1).astype(bfloat16)

    return {
        "wqk": wqk_h, "qkb": qkb, "wvt": wvt_h, "vb": vb_h,
        "wpt": wpt_h, "bproj": inputs["b_proj"].astype(np.float32),
        "w1t": w1t_h, "b1c": b1c_h, "w2t": w2t_h,
        "b2": inputs["b2"].astype(np.float32),
        "ind": _r12(ind), "identb": ident_b, "maskb": maskb, "maskw": maskw,
        "idr": _r12(np.eye(P, dtype=np.float32)),
        "ones1": np.ones(H, np.float32).astype(bfloat16),
        "zeros1": _r12(np.zeros(T, np.float32)),
    }


def _pb(ap, n, extra=None):
    """Prepend a step-0 partition-broadcast dim (+optional middle dims)."""
    dims = [[0, n]] + (extra or []) + list(ap.ap)
    return bass.AP(tensor=ap.tensor, offset=ap.offset, ap=dims)


def _build():
    KPHASE = int(os.environ.get("KPHASE", "5"))
    nc = bacc.Bacc(None, target_bir_lowering=False, debug=False)

    x_d = nc.dram_tensor("x", [BPC, T, D], f32, kind="ExternalInput")
    wqk_d = nc.dram_tensor("wqk", [P, 2, 4, NK, P], f32r, kind="ExternalInput")
    qkb_d = nc.dram_tensor("qkb", [P, 2, 4], f32, kind="ExternalInput")
    wvt_d = nc.dram_tensor("wvt", [P, NK, D], f32r, kind="ExternalInput")
    vb_d = nc.dram_tensor("vb", [D], f32, kind="ExternalInput")
    wpt_d = nc.dram_tensor("wpt", [P, 4, D], f32r, kind="ExternalInput")
    bproj_d = nc.dram_tensor("bproj", [D], f32, kind="ExternalInput")
    w1t_d = nc.dram_tensor("w1t", [P, NK, NM, P], f32r, kind="ExternalInput")
    b1c_d = nc.dram_tensor("b1c", [P, NM], f32, kind="ExternalInput")
    w2t_d = nc.dram_tensor("w2t", [P, NM, D], f32r, kind="ExternalInput")
    b2_d = nc.dram_tensor("b2", [D], f32, kind="ExternalInput")
    ind_d = nc.dram_tensor("ind", [H, 4, P], f32r, kind="ExternalInput")
    identb_d = nc.dram_tensor("identb", [P, P], bf16, kind="ExternalInput")
    maskb_d = nc.dram_tensor("maskb", [P, P], bf16, kind="ExternalInput")
    maskw_d = nc.dram_tensor("maskw", [P, 2 * P], bf16, kind="ExternalInput")
    idr_d = nc.dram_tensor("idr", [P, P], f32r, kind="ExternalInput")
    ones_d = nc.dram_tensor("ones1", [H], bf16, kind="ExternalInput")
    zeros_d = nc.dram_tensor("zeros1", [T], f32r, kind="ExternalInput")
    out_d = nc.dram_tensor("out", [BPC, T, D], f32, kind="ExternalOutput")

    with tile.TileContext(nc) as tc:
        import contextlib
        ctx = contextlib.ExitStack()
        with ctx:
            const = ctx.enter_context(tc.tile_pool(name="const", bufs=1))
            pool1 = ctx.enter_context(tc.tile_pool(name="pool1", bufs=1))
            pool2 = ctx.enter_context(tc.tile_pool(name="pool2", bufs=2))
            pool3 = ctx.enter_context(tc.tile_pool(name="pool3", bufs=3))
            pool_r = ctx.enter_context(tc.tile_pool(name="pool_r", bufs=1))
            ps_sc = ctx.enter_context(tc.tile_pool(name="ps_sc", bufs=2, space="PSUM"))
            ps_av = ctx.enter_context(tc.tile_pool(name="ps_av", bufs=2, space="PSUM"))
            ps_ms = ctx.enter_context(tc.tile_pool(name="ps_ms", bufs=4, space="PSUM"))

            # ---- resident constants ----
            qkb_t = const.tile([P, 2, 4], f32)
            nc.sync.dma_start(out=qkb_t[:, :, :], in_=qkb_d[:, :, :])
            wvt_t = const.tile([P, NK, D], f32r)
            nc.sync.dma_start(out=wvt_t[:, :, :], in_=wvt_d[:, :, :])
            vb_t = const.tile([P, D], f32)
            nc.gpsimd.dma_start(out=vb_t[:, :], in_=_pb(vb_d[:], P))
            wpt_t = const.tile([P, 4, D], f32r)
            nc.sync.dma_start(out=wpt_t[:, :, :], in_=wpt_d[:, :, :])
            bproj_t = const.tile([P, D], f32)
            nc.gpsimd.dma_start(out=bproj_t[:, :], in_=_pb(bproj_d[:], P))
            b1c_t = const.tile([P, NM], f32)
            nc.sync.dma_start(out=b1c_t[:, :], in_=b1c_d[:, :])
            b2_t = const.tile([P, D], f32)
            nc.gpsimd.dma_start(out=b2_t[:, :], in_=_pb(b2_d[:], P))
            ind_t = const.tile([H, 4, P], f32r)
            nc.sync.dma_start(out=ind_t[:, :, :], in_=ind_d[:, :, :])
            identb_t = const.tile([P, P], bf16)
            nc.sync.dma_start(out=identb_t[:, :], in_=identb_d[:, :])
            maskb_t = const.tile([P, P], bf16)
            nc.sync.dma_start(out=maskb_t[:, :], in_=maskb_d[:, :])
            maskw_t = const.tile([P, 2 * P], bf16)
            nc.sync.dma_start(out=maskw_t[:, :], in_=maskw_d[:, :])
            idr_t = const.tile([P, P], f32r)
            nc.sync.dma_start(out=idr_t[:, :], in_=idr_d[:, :])
            eps_t = const.tile([P, 1], f32)
            nc.vector.memset(eps_t[:, :], EPS)
            negc_t = const.tile([P, 1], f32)
            nc.vector.memset(negc_t[:, :], -C)

            def layernorm_to_featT(src_t, tag_prefix):
                """src_t [128, NT, 448] fp32 -> featT [128, NK, 1024] f32r."""
                mv_t = pool3.tile([P, NT, 2], f32, tag="mv")
                for i in range(NT):
                    st = pool3.tile([P, 6], f32, tag="stats")
                    nc.vector.bn_stats(out=st[:, :], in_=src_t[:, i, :])
                    nc.vector.bn_aggr(out=mv_t[:, i, :], in_=st[:, :])
                lg_t = pool3.tile([P, NT], f32, tag="lg")
                nc.scalar.activation(lg_t[:, :], mv_t[:, :, 1], AF.Ln, bias=eps_t[:, 0:1])
                rstd_t = pool3.tile([P, NT], f32, tag="rstd")
                nc.scalar.activation(rstd_t[:, :], lg_t[:, :], AF.Exp, scale=-0.5)

                ft = pool1.tile([P, NK, T], f32r, tag="featT")
                nc.gpsimd.dma_start(
                    out=ft[64:128, 3, :],
                    in_=_pb(zeros_d[:], 64))
                for g in range(2):        # groups of 4 t-tiles
                    for kk in range(NK):
                        w = 128 if kk < 3 else 64
                        pt = ps_ms.tile([P, CH], f32r, tag="ps_ms")
                        for ii in range(4):
                            i = 4 * g + ii
                            h_t = pool3.tile([P, P], f32r, tag="h")
                            nc.vector.tensor_scalar(
                                out=h_t[:, 0:w],
                                in0=src_t[:, i, 128 * kk:128 * kk + w],
                                scalar1=mv_t[:, i, 0:1],
                                scalar2=rstd_t[:, i:i + 1],
                                op0=ALU.subtract, op1=ALU.mult)
                            nc.tensor.transpose(
                                pt[0:w, 128 * ii:128 * ii + 128],
                                h_t[:, 0:w], idr_t[:, :])
                        nc.vector.tensor_copy(
                            ft[0:w, kk, CH * g:CH * g + CH], pt[0:w, :])
                return ft

            for b in range(BPC):
                # ---- load x (also the residual base tile) ----
                xb_t = pool2.tile([P, NT, D], f32, tag="resid")
                nc.sync.dma_start(
                    out=xb_t[:, :, :],
                    in_=x_d[b].rearrange("(n p) d -> p n d", p=P))

                # ---- LN1 + transpose ----
                hT_t = layernorm_to_featT(xb_t, "ln1")

                # xb := x + b_proj (in place; after LN1 reads)
                for i in range(NT):
                    nc.vector.tensor_add(xb_t[:, i, :], xb_t[:, i, :], bproj_t[:, :])

                if KPHASE < 2:
                    continue
                # ---- QKV ----
                wqk_t = pool1.tile([P, 2, 4, NK, P], f32r, tag="shA")
                for qk in range(2):
                    for prx in range(4):
                        nc.sync.dma_start(out=wqk_t[:, qk, prx, :, :],
                                          in_=wqk_d[:, qk, prx, :, :])
                qT_t = pool1.tile([P, 4, T], f32r, tag="shQ")
                kT_t = pool1.tile([P, 4, T], f32r, tag="shK")
                for pr in range(4):
                    for c in range(2):
                        pq = ps_ms.tile([P, CH], f32, tag="ps_ms")
                        for kk in range(NK):
                            nc.tensor.matmul(
                                pq[:, :], wqk_t[:, 0, pr, kk, :],
                                hT_t[:, kk, CH * c:CH * c + CH],
                                start=(kk == 0), stop=(kk == NK - 1))
                        nc.scalar.activation(
                            qT_t[:, pr, CH * c:CH * c + CH], pq[:, :],
                            AF.Identity, bias=qkb_t[:, 0, pr:pr + 1])
                        pk = ps_ms.tile([P, CH], f32, tag="ps_ms")
                        for kk in range(NK):
                            nc.tensor.matmul(
                                pk[:, :], wqk_t[:, 1, pr, kk, :],
                                hT_t[:, kk, CH * c:CH * c + CH],
                                start=(kk == 0), stop=(kk == NK - 1))
                        nc.vector.tensor_scalar(
                            out=kT_t[:, pr, CH * c:CH * c + CH], in0=pk[:, :],
                            scalar1=qkb_t[:, 1, pr:pr + 1], scalar2=None,
                            op0=ALU.add)

                # ---- V (token-major, with ones column) ----
                vext_t = pool1.tile([P, NT, H, 57], bf16, tag="vext")
                nc.gpsimd.dma_start(
                    out=vext_t[:, :, :, 56:57],
                    in_=bass.AP(tensor=ones_d[:].tensor, offset=0,
                                ap=[[0, P], [0, NT * H], [1, 1]]))
                for i in range(NT):
                    pv = ps_ms.tile([P, CH], f32, tag="ps_ms")
                    for kk in range(NK):
                        nc.tensor.matmul(
                            pv[:, 0:D], hT_t[:, kk, 128 * i:128 * i + 128],
                            wvt_t[:, kk, :],
                            start=(kk == 0), stop=(kk == NK - 1))
                    nc.vector.tensor_add(
                        vext_t[:, i, :, 0:56],
                        pv[:, 0:D].rearrange("p (h e) -> p h e", h=H),
                        vb_t[:, :].rearrange("p (h e) -> p h e", h=H))

                if KPHASE < 3:
                    continue
                # ---- attention ----
                attnT_t = pool1.tile([P, 4, T], f32r, tag="shA")
                l_t = pool1.tile([H, T], f32r, tag="l")
                for pr in range(4):
                    for c in range(2):
                        pav = ps_av.tile([P, CH], f32, tag="ps_av")
                        for half in range(2):
                            h = 2 * pr + half
                            poff = 64 * half
                            jmax = 4 * c + 3
                            e_t = pool2.tile([P, 8, CH], bf16, tag="shE")
                            for j in range(jmax + 1):
                                co = max(0, 128 * (j - 4 * c))
                                if co == 384:
                                    co = 256   # keep matmul N >= 256 for f32r speed
                                pss = ps_sc.tile([P, CH], f32, tag="ps_sc")
                                diag = j >= 4 * c
                                nc.tensor.matmul(
                                    pss[:, co:CH],
                                    kT_t[poff:poff + 56, pr, 128 * j:128 * j + 128],
                                    qT_t[poff:poff + 56, pr, CH * c + co:CH * c + CH],
                                    start=True, stop=not diag)
                                if diag:
                                    dco = 128 * (j - 4 * c)
                                    if dco == 384:
                                        nc.tensor.matmul(
                                            pss[:, 256:512],
                                            identb_t[:, :], maskw_t[:, :],
                                            start=False, stop=True)
                                    else:
                                        nc.tensor.matmul(
                                            pss[:, dco:dco + 128],
                                            identb_t[:, :], maskb_t[:, :],
                                            start=False, stop=True)
                                nc.scalar.activation(
                                    e_t[:, j, co:CH], pss[:, co:CH],
                                    AF.Exp, scale=SCALE, bias=negc_t[:, 0:1])
                            for j in range(jmax + 1):
                                co = max(0, 128 * (j - 4 * c))
                                if co == 384:
                                    co = 256
                                nc.tensor.matmul(
                                    pav[poff:poff + 57, co:CH],
                                    vext_t[:, j, h, :], e_t[:, j, co:CH],
                                    tile_position=(0, poff),
                                    start=(j == 0), stop=(j == jmax))
                        nc.scalar.copy(
                            attnT_t[0:57, pr, CH * c:CH * c + CH], pav[0:57, :])
                        nc.vector.tensor_copy(
                            attnT_t[64:121, pr, CH * c:CH * c + CH], pav[64:121, :])
                        for half in range(2):
                            nc.sync.dma_start(
                                out=l_t[2 * pr + half:2 * pr + half + 1,
                                        CH * c:CH * c + CH],
                                in_=attnT_t[56 + 64 * half:57 + 64 * half, pr,
                                            CH * c:CH * c + CH])

                # prefetch MLP weights (slots freed once scores are done)
                w1t_t = pool1.tile([P, NK, NM, P], f32r, tag="shQ")
                for kk in range(NK):
                    nc.sync.dma_start(out=w1t_t[:, kk, :, :], in_=w1t_d[:, kk, :, :])
                w2t_t = pool1.tile([P, NM, D], f32r, tag="shK")
                for g in range(2):
                    nc.sync.dma_start(out=w2t_t[:, 7 * g:7 * g + 7, :],
                                      in_=w2t_d[:, 7 * g:7 * g + 7, :])

                # recip of softmax denominators: 1/l = exp(-log(l))
                nc.scalar.activation(l_t[:, :], l_t[:, :], AF.Ln)
                recip_t = pool1.tile([H, T], f32r, tag="recip")
                nc.scalar.activation(recip_t[:, :], l_t[:, :], AF.Exp, scale=-1.0)

                for pr in range(4):
                    for c in range(2):
                        pb = ps_ms.tile([P, CH], f32, tag="ps_ms")
                        nc.tensor.matmul(
                            pb[:, :], ind_t[:, pr, :],
                            recip_t[:, CH * c:CH * c + CH],
                            start=True, stop=True)
                        nc.vector.tensor_mul(
                            attnT_t[0:57, pr, CH * c:CH * c + CH],
                            attnT_t[0:57, pr, CH * c:CH * c + CH], pb[0:57, :])
                        nc.vector.tensor_mul(
                            attnT_t[64:121, pr, CH * c:CH * c + CH],
                            attnT_t[64:121, pr, CH * c:CH * c + CH], pb[64:121, :])

                if KPHASE < 4:
                    continue
                # ---- proj + residual ----
                x2_t = pool2.tile([P, NT, D], f32, tag="resid")
                for i in range(NT):
                    ppe = ps_ms.tile([P, CH], f32, tag="ps_ms")
                    ppo = ps_ms.tile([P, CH], f32, tag="ps_ms")
                    for half in range(2):
                        poff = 64 * half
                        pp = ppe if half == 0 else ppo
                        for pr in range(4):
                            nc.tensor.matmul(
                                pp[:, 0:D],
                                attnT_t[poff:poff + 56, pr, 128 * i:128 * i + 128],
                                wpt_t[poff:poff + 56, pr, :],
                                start=(pr == 0), stop=(pr == 3))
                    t1_t = pool2.tile([P, D], f32, tag="projtmp")
                    nc.vector.tensor_add(t1_t[:, :], ppe[:, 0:D], xb_t[:, i, :])
                    nc.vector.tensor_add(x2_t[:, i, :], ppo[:, 0:D], t1_t[:, :])

                # ---- LN2 + transpose ----
                h2T_t = layernorm_to_featT(x2_t, "ln2")

                # x2 := x2 + b2 (in place; after LN2 reads)
                for i in range(NT):
                    nc.vector.tensor_add(x2_t[:, i, :], x2_t[:, i, :], b2_t[:, :])

                if KPHASE < 5:
                    continue
                # ---- MLP ----
                for c2 in range(2):          # 512-wide t-chunks
                    rel_t = pool_r.tile([P, NM, CH], f32r, tag="reluT")
                    for m in range(NM):
                        pf = ps_ms.tile([P, CH], f32, tag="ps_ms")
                        for kk in range(NK):
                            nc.tensor.matmul(
                                pf[:, :], w1t_t[:, kk, m, :],
                                h2T_t[:, kk, CH * c2:CH * c2 + CH],
                                start=(kk == 0), stop=(kk == NK - 1))
                        if m % 2 == 0:
                            nc.scalar.activation(
                                rel_t[:, m, :], pf[:, :],
                                AF.Relu, bias=b1c_t[:, m:m + 1])
                        else:
                            nc.vector.tensor_scalar(
                                out=rel_t[:, m, :], in0=pf[:, :],
                                scalar1=b1c_t[:, m:m + 1], scalar2=0.0,
                                op0=ALU.add, op1=ALU.max)
                    for ii in range(4):
                        i = 4 * c2 + ii
                        po = ps_ms.tile([P, CH], f32, tag="ps_ms")
                        for k14 in range(NM):
                            nc.tensor.matmul(
                                po[:, 0:D],
                                rel_t[:, k14, 128 * ii:128 * ii + 128],
                                w2t_t[:, k14, :],
                                start=(k14 == 0), stop=(k14 == NM - 1))
                        o_t = pool2.tile([P, D], f32, tag="outt")
                        nc.vector.tensor_add(o_t[:, :], po[:, 0:D], x2_t[:, i, :])
                        nc.sync.dma_start(
                            out=out_d[b, 128 * i:128 * i + 128, :], in_=o_t[:, :])

    nc.finalize()
    return nc


_CACHE = {}


def run(inputs, trace=False):
    if "nc" not in _CACHE:
        _CACHE["nc"] = _build()
    nc = _CACHE["nc"]
    host = _prep({k: np.asarray(v) for k, v in inputs.items()})
    x = np.asarray(inputs["x"], np.float32)
    in_maps = []
    for cidx in range(NCORE):
        m = dict(host)
        m["x"] = np.ascontiguousarray(x[BPC * cidx:BPC * cidx + BPC])
        in_maps.append(m)
    r = run_bass_kernel_spmd(nc, in_maps, list(range(NCORE)), trace=trace)
    out = np.concatenate([r.results[cidx]["out"] for cidx in range(NCORE)], axis=0)
    return out, r


def kernel(**inputs):
    out, _ = run(inputs, trace=False)
    return out


# revision 20
# speedup vs baseline: 1.0299x; 1.0048x over previous
"""Trainium2 Bass kernel for nn_Block (dense transformer block).

Sharding: pure data-parallel over batch — 16 batch elements, 2 per core,
no collectives. Each core runs the full block on its 2 batch elements.

Design (per core, per batch element):
  - LN1 in token-major layout (free-dim stats via bn_stats/bn_aggr;
    rstd = exp(-0.5*log(var+eps)) so only the Exp/Log ACT table set is used)
  - h transposed to feature-major hT [D, T] via PE transposes
  - Q/K computed feature-major (heads packed in pairs at partition offsets
    {0, 64}); V computed token-major with a per-head ones column appended
  - scores computed DIRECTLY transposed: sT[s, t] = kT.T @ qT, so softmax
    exp needs only a global constant bias (-C), no per-row max
  - causal masking: matmul N-range restriction (left of diagonal) + a bf16
    identity@mask matmul adding -1e9 on the diagonal blocks
  - AV feature-major: attnT[e, t] = v_ext.T @ expT; the ones column yields
    the softmax denominators l as rows 56/120 of the psum
  - per-head 1/l: l rows gathered via tiny SBUF->SBUF DMAs, recip via ACT
    Log+Exp, broadcast back to 128 partitions via an indicator matmul,
    applied as one in-place tensor_tensor multiply
  - proj token-major (lhsT = attnT slices; heads at offsets {0,64} packed
    into K via tile_position row groups), +residual +b_proj
  - LN2, fc1 feature-major + relu (+b1), fc2 token-major (+b2) + residual

All matmuls run in float32r (fp32 rounded to 11-bit mantissa, RNE —
verified bit-exact against HW), which streams at 1 cycle/row like bf16.
LN gains/biases are folded into adjacent matmul weights host-side.
"""
import sys, os
sys.path.insert(0, "/opt/trn_rl_repo")
import math
import numpy as np

import concourse.bass as bass
import concourse.bacc as bacc
import concourse.tile as tile
from concourse import mybir
from concourse.bass_utils import run_bass_kernel_spmd
from ml_dtypes import bfloat16

P = 128
B, T, D, H = 16, 1024, 448, 8
HS = D // H            # 56
DH = 4 * D             # 1792
NT = T // P            # 8 t-tiles per batch element
NK = 4                 # d k-tiles (448 padded to 512)
NM = DH // P           # 14
NCORE = 8
BPC = B // NCORE       # 2 batch elements per core
EPS = 1e-5
SCALE = 1.0 / math.sqrt(D)
C = 4.0                # global exp bias; max |scaled score| measured 0.575
NEG = -1.0e9
CH = 512               # attention t-chunk
F2 = 256               # mlp t-chunk

f32 = mybir.dt.float32
f32r = mybir.dt.float32r
bf16 = mybir.dt.bfloat16
AF = mybir.ActivationFunctionType
ALU = mybir.AluOpType


def _r12(a):
    """f32r rounding: RNE to 11-bit mantissa (bit-exact vs HW)."""
    b = np.ascontiguousarray(np.asarray(a, np.float32)).view(np.uint32).astype(np.uint64)
    r = (b + 0x7FF + ((b >> 12) & 1)) & np.uint64(0xFFFFF000)
    return r.astype(np.uint32).view(np.float32)


def _prep(inputs):
    """Host-side weight packing/folding. Returns dict of shared arrays."""
    wq, wk, wv = inputs["wq"], inputs["wk"], inputs["wv"]
    g1, bb1 = inputs["ln1_g"], inputs["ln1_b"]
    g2, bb2 = inputs["ln2_g"], inputs["ln2_b"]
    w_proj, w1, w2 = inputs["w_proj"], inputs["w1"], inputs["w2"]
    b1 = inputs["b1"]

    wq_e = (wq * g1[None, None, :]).astype(np.float32)   # [H, HS, D]
    wk_e = (wk * g1[None, None, :]).astype(np.float32)
    wv_e = (wv * g1[None, None, :]).astype(np.float32)
    qbias = np.einsum('hed,d->he', wq_e, bb1).astype(np.float32)
    kbias = np.einsum('hed,d->he', wk_e, bb1).astype(np.float32)
    vbias = np.einsum('hed,d->he', wv_e, bb1).astype(np.float32)

    # Q/K lhsT: [p, qk, pair, ktile, col] — head 2p at cols 0-55, 2p+1 at 64-119
    wqk = np.zeros((2, 4, NK, P, P), np.float32)
    for qk, W in enumerate((wq_e, wk_e)):
        for pr in range(4):
            for kk in range(NK):
                lo, hi = 128 * kk, min(128 * kk + 128, D)
                n = hi - lo
                wqk[qk, pr, kk, :n, 0:56] = W[2 * pr][:, lo:hi].T
                wqk[qk, pr, kk, :n, 64:120] = W[2 * pr + 1][:, lo:hi].T
    wqk_h = wqk.transpose(3, 0, 1, 2, 4).copy().astype(bfloat16)  # [128, 2, 4, 4, 128]

    qkb = np.zeros((P, 2, 4), np.float32)                 # bias cols for q/k evict
    for qk, bias in enumerate((qbias, kbias)):
        for pr in range(4):
            qkb[0:56, qk, pr] = bias[2 * pr]
            qkb[64:120, qk, pr] = bias[2 * pr + 1]

    # V rhs: [ktile, row, outfeat 448] head-major; vbias row in [8,56] layout
    wvt = np.zeros((NK, P, D), np.float32)
    for kk in range(NK):
        lo, hi = 128 * kk, min(128 * kk + 128, D)
        for h in range(H):
            wvt[kk, :hi - lo, 56 * h:56 * h + 56] = wv_e[h][:, lo:hi].T
    wvt_h = wvt.transpose(1, 0, 2).copy().astype(bfloat16)  # [128, 4, 448]
    vb_h = vbias.reshape(D).astype(np.float32)            # [448] head-major

    # proj rhs: rows packed to match attnT (head 2p at 0-55, 2p+1 at 64-119)
    wpT = w_proj.T.astype(np.float32)                     # [in, out]
    wpt = np.zeros((4, P, D), np.float32)
    for pr in range(4):
        wpt[pr, 0:56, :] = wpT[56 * (2 * pr):56 * (2 * pr) + 56, :]
        wpt[pr, 64:120, :] = wpT[56 * (2 * pr + 1):56 * (2 * pr + 1) + 56, :]
    wpt_h = _r12(wpt.transpose(1, 0, 2).copy())           # [128, 4, 448]

    # fc1 lhsT tiles [ktile, mtile, row, col]; fc2 rhs tiles [ktile, row, 448]
    w1_e = (w1 * g2[None, :]).astype(np.float32)          # [DH, D]
    b1_e = (b1 + w1_e @ bb2).astype(np.float32)
    w1T = np.zeros((512, DH), np.float32)
    w1T[:D] = w1_e.T
    w1t_h = w1T.reshape(NK, P, NM, P).transpose(1, 0, 2, 3).copy().astype(bfloat16)
    b1c_h = b1_e.reshape(NM, P).T.copy()                  # [128, 14]
    w2t_h = w2.T.reshape(NM, P, D).transpose(1, 0, 2).copy().astype(bfloat16)

    # indicator for per-head recip broadcast: [8, pair, 128]
    ind = np.zeros((H, 4, P), np.float32)
    for pr in range(4):
        ind[2 * pr, pr, 0:56] = 1.0
        ind[2 * pr + 1, pr, 64:120] = 1.0

    # bf16 identity + causal mask (in [s_local, t_local]: mask s>t)
    ident_b = np.eye(P, dtype=np.float32).astype(bfloat16)
    tri = np.where(np.arange(P)[:, None] > np.arange(P)[None, :],
                   np.float32(NEG), np.float32(0.0))
    maskb = tri.astype(bfloat16)
    maskw = np.concatenate(
        [np.full((P, P), np.float32(NEG)), tri], axis=1).astype(bfloat16)

    return {
        "wqk": wqk_h, "qkb": qkb, "wvt": wvt_h, "vb": vb_h,
        "wpt": wpt_h, "bproj": inputs["b_proj"].astype(np.float32),
        "w1t": w1t_h, "b1c": b1c_h, "w2t": w2t_h,
        "b2": inputs["b2"].astype(np.float32),
        "ind": _r12(ind), "identb": ident_b, "maskb": maskb, "maskw": maskw,
        "ones1": np.ones(H, np.float32).astype(bfloat16),
        "zeros1": np.zeros(T, np.float32).astype(bfloat16),
    }


def _pb(ap, n, extra=None):
    """Prepend a step-0 partition-broadcast dim (+optional middle dims)."""
    dims = [[0, n]] + (extra or []) + list(ap.ap)
    return bass.AP(tensor=ap.tensor, offset=ap.offset, ap=dims)


def _build():
    KPHASE = int(os.environ.get("KPHASE", "5"))
    nc = bacc.Bacc(None, target_bir_lowering=False, debug=False)

    x_d = nc.dram_tensor("x", [BPC, T, D], f32, kind="ExternalInput")
    wqk_d = nc.dram_tensor("wqk", [P, 2, 4, NK, P], bf16, kind="ExternalInput")
    qkb_d = nc.dram_tensor("qkb", [P, 2, 4], f32, kind="ExternalInput")
    wvt_d = nc.dram_tensor("wvt", [P, NK, D], bf16, kind="ExternalInput")
    vb_d = nc.dram_tensor("vb", [D], f32, kind="ExternalInput")
    wpt_d = nc.dram_tensor("wpt", [P, 4, D], f32r, kind="ExternalInput")
    bproj_d = nc.dram_tensor("bproj", [D], f32, kind="ExternalInput")
    w1t_d = nc.dram_tensor("w1t", [P, NK, NM, P], bf16, kind="ExternalInput")
    b1c_d = nc.dram_tensor("b1c", [P, NM], f32, kind="ExternalInput")
    w2t_d = nc.dram_tensor("w2t", [P, NM, D], bf16, kind="ExternalInput")
    b2_d = nc.dram_tensor("b2", [D], f32, kind="ExternalInput")
    ind_d = nc.dram_tensor("ind", [H, 4, P], f32r, kind="ExternalInput")
    identb_d = nc.dram_tensor("identb", [P, P], bf16, kind="ExternalInput")
    maskb_d = nc.dram_tensor("maskb", [P, P], bf16, kind="ExternalInput")
    maskw_d = nc.dram_tensor("maskw", [P, 2 * P], bf16, kind="ExternalInput")
    ones_d = nc.dram_tensor("ones1", [H], bf16, kind="ExternalInput")
    zeros_d = nc.dram_tensor("zeros1", [T], bf16, kind="ExternalInput")
    out_d = nc.dram_tensor("out", [BPC, T, D], f32, kind="ExternalOutput")

    with tile.TileContext(nc) as tc:
        import contextlib
        ctx = contextlib.ExitStack()
        with ctx:
            const = ctx.enter_context(tc.tile_pool(name="const", bufs=1))
            pool1 = ctx.enter_context(tc.tile_pool(name="pool1", bufs=1))
            pool2 = ctx.enter_context(tc.tile_pool(name="pool2", bufs=2))
            pool3 = ctx.enter_context(tc.tile_pool(name="pool3", bufs=3))
            pool_r = ctx.enter_context(tc.tile_pool(name="pool_r", bufs=1))
            pool_e = ctx.enter_context(tc.tile_pool(name="pool_e", bufs=3))
            ps_sc = ctx.enter_context(tc.tile_pool(name="ps_sc", bufs=2, space="PSUM"))
            ps_av = ctx.enter_context(tc.tile_pool(name="ps_av", bufs=2, space="PSUM"))
            ps_ms = ctx.enter_context(tc.tile_pool(name="ps_ms", bufs=4, space="PSUM"))

            # ---- resident constants ----
            qkb_t = const.tile([P, 2, 4], f32)
            nc.sync.dma_start(out=qkb_t[:, :, :], in_=qkb_d[:, :, :])
            wvt_t = const.tile([P, NK, D], bf16)
            nc.sync.dma_start(out=wvt_t[:, :, :], in_=wvt_d[:, :, :])
            vb_t = const.tile([P, D], f32)
            nc.gpsimd.dma_start(out=vb_t[:, :], in_=_pb(vb_d[:], P))
            wpt_t = const.tile([P, 4, D], f32r)
            nc.sync.dma_start(out=wpt_t[:, :, :], in_=wpt_d[:, :, :])
            bproj_t = const.tile([P, D], f32)
            nc.gpsimd.dma_start(out=bproj_t[:, :], in_=_pb(bproj_d[:], P))
            b1c_t = const.tile([P, NM], f32)
            nc.sync.dma_start(out=b1c_t[:, :], in_=b1c_d[:, :])
            b2_t = const.tile([P, D], f32)
            nc.gpsimd.dma_start(out=b2_t[:, :], in_=_pb(b2_d[:], P))
            ind_t = const.tile([H, 4, P], f32r)
            nc.sync.dma_start(out=ind_t[:, :, :], in_=ind_d[:, :, :])
            identb_t = const.tile([P, P], bf16)
            nc.sync.dma_start(out=identb_t[:, :], in_=identb_d[:, :])
            maskb_t = const.tile([P, P], bf16)
            nc.sync.dma_start(out=maskb_t[:, :], in_=maskb_d[:, :])
            maskw_t = const.tile([P, 2 * P], bf16)
            nc.sync.dma_start(out=maskw_t[:, :], in_=maskw_d[:, :])
            w1t_t = const.tile([P, NK, NM, P], bf16)
            w2t_t = const.tile([P, NM, D], bf16)
            eps_t = const.tile([P, 1], f32)
            nc.vector.memset(eps_t[:, :], EPS)
            negc_t = const.tile([P, 1], f32)
            nc.vector.memset(negc_t[:, :], -C)

            def layernorm_to_featT(src_t, tag_prefix):
                """src_t [128, NT, 448] fp32 -> featT [128, NK, 1024] f32r."""
                mv_t = pool3.tile([P, NT, 2], f32, tag="mv")
                for i in range(NT):
                    st = pool3.tile([P, 6], f32, tag="stats")
                    nc.vector.bn_stats(out=st[:, :], in_=src_t[:, i, :])
                    nc.vector.bn_aggr(out=mv_t[:, i, :], in_=st[:, :])
                lg_t = pool3.tile([P, NT], f32, tag="lg")
                nc.scalar.activation(lg_t[:, :], mv_t[:, :, 1], AF.Ln, bias=eps_t[:, 0:1])
                rstd_t = pool3.tile([P, NT], f32, tag="rstd")
                nc.scalar.activation(rstd_t[:, :], lg_t[:, :], AF.Exp, scale=-0.5)

                ft = pool1.tile([P, NK, T], bf16, tag="featT")
                nc.gpsimd.dma_start(
                    out=ft[64:128, 3, :],
                    in_=_pb(zeros_d[:], 64))
                for g in range(2):        # groups of 4 t-tiles
                    for kk in range(NK):
                        w = 128 if kk < 3 else 64
                        pt = ps_ms.tile([P, CH], bf16, tag="ps_ms")
                        for ii in range(4):
                            i = 4 * g + ii
                            h_t = pool3.tile([P, P], bf16, tag="h")
                            nc.vector.tensor_scalar(
                                out=h_t[:, 0:w],
                                in0=src_t[:, i, 128 * kk:128 * kk + w],
                                scalar1=mv_t[:, i, 0:1],
                                scalar2=rstd_t[:, i:i + 1],
                                op0=ALU.subtract, op1=ALU.mult)
                            nc.tensor.transpose(
                                pt[0:w, 128 * ii:128 * ii + 128],
                                h_t[:, 0:w], identb_t[:, :])
                        nc.vector.tensor_copy(
                            ft[0:w, kk, CH * g:CH * g + CH], pt[0:w, :])
                return ft

            for b in range(BPC):
                # ---- load x (also the residual base tile) ----
                xb_t = pool2.tile([P, NT, D], f32, tag="resid")
                nc.sync.dma_start(
                    out=xb_t[:, :, :],
                    in_=x_d[b].rearrange("(n p) d -> p n d", p=P))

                # ---- LN1 + transpose ----
                hT_t = layernorm_to_featT(xb_t, "ln1")

                # xb := x + b_proj (in place; after LN1 reads)
                for i in range(NT):
                    nc.vector.tensor_add(xb_t[:, i, :], xb_t[:, i, :], bproj_t[:, :])

                if KPHASE < 2:
                    continue
                # ---- QKV ----
                wqk_t = pool1.tile([P, 2, 4, NK, P], bf16, tag="shA")
                for qk in range(2):
                    for prx in range(4):
                        nc.sync.dma_start(out=wqk_t[:, qk, prx, :, :],
                                          in_=wqk_d[:, qk, prx, :, :])
                qT_t = pool1.tile([P, 4, T], bf16, tag="shQ")
                kT_t = pool1.tile([P, 4, T], bf16, tag="shK")
                for pr in range(4):
                    for c in range(2):
                        pq = ps_ms.tile([P, CH], f32, tag="ps_ms")
                        for kk in range(NK):
                            nc.tensor.matmul(
                                pq[:, :], wqk_t[:, 0, pr, kk, :],
                                hT_t[:, kk, CH * c:CH * c + CH],
                                start=(kk == 0), stop=(kk == NK - 1))
                        nc.scalar.activation(
                            qT_t[:, pr, CH * c:CH * c + CH], pq[:, :],
                            AF.Identity, bias=qkb_t[:, 0, pr:pr + 1])
                        pk = ps_ms.tile([P, CH], f32, tag="ps_ms")
                        for kk in range(NK):
                            nc.tensor.matmul(
                                pk[:, :], wqk_t[:, 1, pr, kk, :],
                                hT_t[:, kk, CH * c:CH * c + CH],
                                start=(kk == 0), stop=(kk == NK - 1))
                        nc.vector.tensor_scalar(
                            out=kT_t[:, pr, CH * c:CH * c + CH], in0=pk[:, :],
                            scalar1=qkb_t[:, 1, pr:pr + 1], scalar2=None,
                            op0=ALU.add)

                if b == 0:
                    for kk in range(NK):
                        nc.sync.dma_start(out=w1t_t[:, kk, :, :],
                                          in_=w1t_d[:, kk, :, :])
                    for g in range(2):
                        nc.sync.dma_start(out=w2t_t[:, 7 * g:7 * g + 7, :],
                                          in_=w2t_d[:, 7 * g:7 * g + 7, :])

                # ---- V (token-major, with ones column) ----
                vext_t = pool1.tile([P, NT, H, 57], bf16, tag="vext")
                nc.gpsimd.dma_start(
                    out=vext_t[:, :, :, 56:57],
                    in_=bass.AP(tensor=ones_d[:].tensor, offset=0,
                                ap=[[0, P], [0, NT * H], [1, 1]]))
                for i in range(NT):
                    pv = ps_ms.tile([P, CH], f32, tag="ps_ms")
                    for kk in range(NK):
                        nc.tensor.matmul(
                            pv[:, 0:D], hT_t[:, kk, 128 * i:128 * i + 128],
                            wvt_t[:, kk, :],
                            start=(kk == 0), stop=(kk == NK - 1))
                    nc.vector.tensor_add(
                        vext_t[:, i, :, 0:56],
                        pv[:, 0:D].rearrange("p (h e) -> p h e", h=H),
                        vb_t[:, :].rearrange("p (h e) -> p h e", h=H))

                if KPHASE < 3:
                    continue
                # ---- attention ----
                attnT_t = pool1.tile([P, 4, T], f32r, tag="shA")
                l_t = pool1.tile([H, T], f32r, tag="l")
                for pr in range(4):
                    for c in range(2):
                        pav = ps_av.tile([P, CH], f32, tag="ps_av")
                        for half in range(2):
                            h = 2 * pr + half
                            poff = 64 * half
                            jmax = 4 * c + 3
                            e_t = pool_e.tile([P, 8, CH], bf16, tag="shE")
                            for j in range(jmax + 1):
                                co = max(0, 128 * (j - 4 * c))
                                if co == 384:
                                    co = 256   # keep matmul N >= 256 for f32r speed
                                pss = ps_sc.tile([P, CH], f32, tag="ps_sc")
                                diag = j >= 4 * c
                                nc.tensor.matmul(
                                    pss[:, co:CH],
                                    kT_t[poff:poff + 56, pr, 128 * j:128 * j + 128],
                                    qT_t[poff:poff + 56, pr, CH * c + co:CH * c + CH],
                                    start=True, stop=not diag)
                                if diag:
                                    dco = 128 * (j - 4 * c)
                                    if dco == 384:
                                        nc.tensor.matmul(
                                            pss[:, 256:512],
                                            identb_t[:, :], maskw_t[:, :],
                                            start=False, stop=True)
                                    else:
                                        nc.tensor.matmul(
                                            pss[:, dco:dco + 128],
                                            identb_t[:, :], maskb_t[:, :],
                                            start=False, stop=True)
                                nc.scalar.activation(
                                    e_t[:, j, co:CH], pss[:, co:CH],
                                    AF.Exp, scale=SCALE, bias=negc_t[:, 0:1])
                            for j in range(jmax + 1):
                                co = max(0, 128 * (j - 4 * c))
                                if co == 384:
                                    co = 256
                                nc.tensor.matmul(
                                    pav[poff:poff + 57, co:CH],
                                    vext_t[:, j, h, :], e_t[:, j, co:CH],
                                    tile_position=(0, poff),
                                    start=(j == 0), stop=(j == jmax))
                        nc.scalar.copy(
                            attnT_t[0:57, pr, CH * c:CH * c + CH], pav[0:57, :])
                        nc.vector.tensor_copy(
                            attnT_t[64:121, pr, CH * c:CH * c + CH], pav[64:121, :])
                        for half in range(2):
                            nc.gpsimd.dma_start(
                                out=l_t[2 * pr + half:2 * pr + half + 1,
                                        CH * c:CH * c + CH],
                                in_=attnT_t[56 + 64 * half:57 + 64 * half, pr,
                                            CH * c:CH * c + CH])

                # recip of softmax denominators: 1/l = exp(-log(l))
                nc.scalar.activation(l_t[:, :], l_t[:, :], AF.Ln)
                nc.scalar.activation(l_t[:, :], l_t[:, :], AF.Exp, scale=-1.0)

                for pr in range(4):
                    for c in range(2):
                        pb = ps_ms.tile([P, CH], f32, tag="ps_ms")
                        nc.tensor.matmul(
                            pb[:, :], ind_t[:, pr, :],
                            l_t[:, CH * c:CH * c + CH],
                            start=True, stop=True)
                        nc.vector.tensor_mul(
                            attnT_t[0:57, pr, CH * c:CH * c + CH],
                            attnT_t[0:57, pr, CH * c:CH * c + CH], pb[0:57, :])
                        nc.vector.tensor_mul(
                            attnT_t[64:121, pr, CH * c:CH * c + CH],
                            attnT_t[64:121, pr, CH * c:CH * c + CH], pb[64:121, :])

                if KPHASE < 4:
                    continue
                # ---- proj + residual ----
                x2_t = pool2.tile([P, NT, D], f32, tag="resid")
                for i in range(NT):
                    ppe = ps_ms.tile([P, CH], f32, tag="ps_ms")
                    ppo = ps_ms.tile([P, CH], f32, tag="ps_ms")
                    for half in range(2):
                        poff = 64 * half
                        pp = ppe if half == 0 else ppo
                        for pr in range(4):
                            nc.tensor.matmul(
                                pp[:, 0:D],
                                attnT_t[poff:poff + 56, pr, 128 * i:128 * i + 128],
                                wpt_t[poff:poff + 56, pr, :],
                                start=(pr == 0), stop=(pr == 3))
                    t1_t = pool2.tile([P, D], f32, tag="projtmp")
                    nc.vector.tensor_add(t1_t[:, :], ppe[:, 0:D], xb_t[:, i, :])
                    nc.vector.tensor_add(x2_t[:, i, :], ppo[:, 0:D], t1_t[:, :])

                # ---- LN2 + transpose ----
                h2T_t = layernorm_to_featT(x2_t, "ln2")

                # x2 := x2 + b2 (in place; after LN2 reads)
                for i in range(NT):
                    nc.vector.tensor_add(x2_t[:, i, :], x2_t[:, i, :], b2_t[:, :])

                if KPHASE < 5:
                    continue
                # ---- MLP ----
                for c2 in range(2):          # 512-wide t-chunks
                    rel_t = pool_r.tile([P, NM, CH], bf16, tag="reluT")
                    for m in range(NM):
                        pf = ps_ms.tile([P, CH], f32, tag="ps_ms")
                        for kk in range(NK):
                            nc.tensor.matmul(
                                pf[:, :], w1t_t[:, kk, m, :],
                                h2T_t[:, kk, CH * c2:CH * c2 + CH],
                                start=(kk == 0), stop=(kk == NK - 1))
                        if m % 2 == 0:
                            nc.scalar.activation(
                                rel_t[:, m, :], pf[:, :],
                                AF.Relu, bias=b1c_t[:, m:m + 1])
                        else:
                            nc.vector.tensor_scalar(
                                out=rel_t[:, m, :], in0=pf[:, :],
                                scalar1=b1c_t[:, m:m + 1], scalar2=0.0,
                                op0=ALU.add, op1=ALU.max)
                    for ii in range(4):
                        i = 4 * c2 + ii
                        po = ps_ms.tile([P, CH], f32, tag="ps_ms")
                        for k14 in range(NM):
                            nc.tensor.matmul(
                                po[:, 0:D],
                                rel_t[:, k14, 128 * ii:128 * ii + 128],
                                w2t_t[:, k14, :],
                                start=(k14 == 0), stop=(k14 == NM - 1))
                        o_t = pool2.tile([P, D], f32, tag="outt")
                        nc.vector.tensor_add(o_t[:, :], po[:, 0:D], x2_t[:, i, :])
                        nc.gpsimd.dma_start(
                            out=out_d[b, 128 * i:128 * i + 128, :], in_=o_t[:, :])

    nc.finalize()
    return nc


_CACHE = {}


def run(inputs, trace=False):
    if "nc" not in _CACHE:
        _CACHE["nc"] = _build()
    nc = _CACHE["nc"]
    host = _prep({k: np.asarray(v) for k, v in inputs.items()})
    x = np.asarray(inputs["x"], np.float32)
    in_maps = []
    for cidx in range(NCORE):
        m = dict(host)
        m["x"] = np.ascontiguousarray(x[BPC * cidx:BPC * cidx + BPC])
        in_maps.append(m)
    r = run_bass_kernel_spmd(nc, in_maps, list(range(NCORE)), trace=trace)
    out = np.concatenate([r.results[cidx]["out"] for cidx in range(NCORE)], axis=0)
    return out, r


def kernel(**inputs):
    out, _ = run(inputs, trace=False)
    return out


# revision 25
# speedup vs baseline: 1.1210x; 1.0884x over previous
"""Trainium2 Bass kernel for nn_Block (dense transformer block).

Sharding: pure data-parallel over batch — 16 batch elements, 2 per core,
no collectives. Each core runs the full block on its 2 batch elements.

Design (per core, per batch element):
  - LN1 in token-major layout (free-dim stats via bn_stats/bn_aggr;
    rstd = exp(-0.5*log(var+eps)) so only the Exp/Log ACT table set is used)
  - h transposed to feature-major hT [D, T] via PE transposes
  - Q/K computed feature-major (heads packed in pairs at partition offsets
    {0, 64}); V computed token-major with a per-head ones column appended
  - scores computed DIRECTLY transposed: sT[s, t] = kT.T @ qT, so softmax
    exp needs only a global constant bias (-C), no per-row max
  - causal masking: matmul N-range restriction (left of diagonal) + a bf16
    identity@mask matmul adding -1e9 on the diagonal blocks
  - AV feature-major: attnT[e, t] = v_ext.T @ expT; the ones column yields
    the softmax denominators l as rows 56/120 of the psum
  - per-head 1/l: l rows gathered via tiny SBUF->SBUF DMAs, recip via ACT
    Log+Exp, broadcast back to 128 partitions via an indicator matmul,
    applied as one in-place tensor_tensor multiply
  - proj token-major (lhsT = attnT slices; heads at offsets {0,64} packed
    into K via tile_position row groups), +residual +b_proj
  - LN2, fc1 feature-major + relu (+b1), fc2 token-major (+b2) + residual

All matmuls run in float32r (fp32 rounded to 11-bit mantissa, RNE —
verified bit-exact against HW), which streams at 1 cycle/row like bf16.
LN gains/biases are folded into adjacent matmul weights host-side.
"""
import sys, os
sys.path.insert(0, "/opt/trn_rl_repo")
import math
import numpy as np

import concourse.bass as bass
import concourse.bacc as bacc
import concourse.tile as tile
from concourse import mybir
from concourse.bass_utils import run_bass_kernel_spmd
from ml_dtypes import bfloat16

P = 128
B, T, D, H = 16, 1024, 448, 8
HS = D // H            # 56
DH = 4 * D             # 1792
NT = T // P            # 8 t-tiles per batch element
NK = 4                 # d k-tiles (448 padded to 512)
NM = DH // P           # 14
NCORE = 8
BPC = B // NCORE       # 2 batch elements per core
EPS = 1e-5
SCALE = 1.0 / math.sqrt(D)
C = 4.0                # global exp bias; max |scaled score| measured 0.575
NEG = -1.0e9
CH = 512               # attention t-chunk
F2 = 256               # mlp t-chunk

f32 = mybir.dt.float32
f32r = mybir.dt.float32r
bf16 = mybir.dt.bfloat16
AF = mybir.ActivationFunctionType
ALU = mybir.AluOpType


def _r12(a):
    """f32r rounding: RNE to 11-bit mantissa (bit-exact vs HW)."""
    b = np.ascontiguousarray(np.asarray(a, np.float32)).view(np.uint32).astype(np.uint64)
    r = (b + 0x7FF + ((b >> 12) & 1)) & np.uint64(0xFFFFF000)
    return r.astype(np.uint32).view(np.float32)


def _prep(inputs):
    """Host-side weight packing/folding. Returns dict of shared arrays."""
    wq, wk, wv = inputs["wq"], inputs["wk"], inputs["wv"]
    g1, bb1 = inputs["ln1_g"], inputs["ln1_b"]
    g2, bb2 = inputs["ln2_g"], inputs["ln2_b"]
    w_proj, w1, w2 = inputs["w_proj"], inputs["w1"], inputs["w2"]
    b1 = inputs["b1"]

    wq_e = (wq * g1[None, None, :]).astype(np.float32)   # [H, HS, D]
    wk_e = (wk * g1[None, None, :]).astype(np.float32)
    wv_e = (wv * g1[None, None, :]).astype(np.float32)
    qbias = np.einsum('hed,d->he', wq_e, bb1).astype(np.float32)
    kbias = np.einsum('hed,d->he', wk_e, bb1).astype(np.float32)
    vbias = np.einsum('hed,d->he', wv_e, bb1).astype(np.float32)

    # Q/K lhsT: [p, qk, pair, ktile, col] — head 2p at cols 0-55, 2p+1 at 64-119
    wqk = np.zeros((2, 4, NK, P, P), np.float32)
    for qk, W in enumerate((wq_e, wk_e)):
        for pr in range(4):
            for kk in range(NK):
                lo, hi = 128 * kk, min(128 * kk + 128, D)
                n = hi - lo
                wqk[qk, pr, kk, :n, 0:56] = W[2 * pr][:, lo:hi].T
                wqk[qk, pr, kk, :n, 64:120] = W[2 * pr + 1][:, lo:hi].T
    wqk_h = wqk.transpose(3, 0, 1, 2, 4).copy().astype(bfloat16)  # [128, 2, 4, 4, 128]

    qkb = np.zeros((P, 2, 4), np.float32)                 # bias cols for q/k evict
    for qk, bias in enumerate((qbias, kbias)):
        for pr in range(4):
            qkb[0:56, qk, pr] = bias[2 * pr]
            qkb[64:120, qk, pr] = bias[2 * pr + 1]

    # V rhs: [ktile, row, outfeat 448] head-major; vbias row in [8,56] layout
    wvt = np.zeros((NK, P, D), np.float32)
    for kk in range(NK):
        lo, hi = 128 * kk, min(128 * kk + 128, D)
        for h in range(H):
            wvt[kk, :hi - lo, 56 * h:56 * h + 56] = wv_e[h][:, lo:hi].T
    wvt_h = wvt.transpose(1, 0, 2).copy().astype(bfloat16)  # [128, 4, 448]
    vb_h = vbias.reshape(D).astype(np.float32)            # [448] head-major

    # proj rhs: rows packed to match attnT (head 2p at 0-55, 2p+1 at 64-119)
    wpT = w_proj.T.astype(np.float32)                     # [in, out]
    wpt = np.zeros((4, P, D), np.float32)
    for pr in range(4):
        wpt[pr, 0:56, :] = wpT[56 * (2 * pr):56 * (2 * pr) + 56, :]
        wpt[pr, 64:120, :] = wpT[56 * (2 * pr + 1):56 * (2 * pr + 1) + 56, :]
    wpt_h = _r12(wpt.transpose(1, 0, 2).copy())           # [128, 4, 448]

    # fc1 lhsT tiles [ktile, mtile, row, col]; fc2 rhs tiles [ktile, row, 448]
    w1_e = (w1 * g2[None, :]).astype(np.float32)          # [DH, D]
    b1_e = (b1 + w1_e @ bb2).astype(np.float32)
    w1T = np.zeros((512, DH), np.float32)
    w1T[:D] = w1_e.T
    w1t_h = w1T.reshape(NK, P, NM, P).transpose(1, 0, 2, 3).copy().astype(bfloat16)
    b1c_h = b1_e.reshape(NM, P).T.copy()                  # [128, 14]
    w2t_h = w2.T.reshape(NM, P, D).transpose(1, 0, 2).copy().astype(bfloat16)

    # indicator for per-head recip broadcast: [8, pair, 128]
    ind = np.zeros((H, 4, P), np.float32)
    for pr in range(4):
        ind[2 * pr, pr, 0:56] = 1.0
        ind[2 * pr + 1, pr, 64:120] = 1.0

    # bf16 identity + causal mask (in [s_local, t_local]: mask s>t)
    ident_b = np.eye(P, dtype=np.float32).astype(bfloat16)
    tri = np.where(np.arange(P)[:, None] > np.arange(P)[None, :],
                   np.float32(NEG), np.float32(0.0))
    maskb = tri.astype(bfloat16)
    maskw = np.concatenate(
        [np.full((P, P), np.float32(NEG)), tri], axis=1).astype(bfloat16)

    return {
        "wqk": wqk_h.view(np.uint32), "qkb": qkb, "wvt": wvt_h.view(np.uint32), "vb": vb_h,
        "wpt": wpt_h, "bproj": inputs["b_proj"].astype(np.float32),
        "w1t": w1t_h.view(np.uint32), "b1c": b1c_h, "w2t": w2t_h.view(np.uint32),
        "b2": inputs["b2"].astype(np.float32),
        "ind": _r12(ind), "identb": ident_b, "maskb": maskb, "maskw": maskw,
        "ones1": np.ones(H, np.float32).astype(bfloat16),
        "zeros1": np.zeros(T, np.float32).astype(bfloat16),
    }


def _pb(ap, n, extra=None):
    """Prepend a step-0 partition-broadcast dim (+optional middle dims)."""
    dims = [[0, n]] + (extra or []) + list(ap.ap)
    return bass.AP(tensor=ap.tensor, offset=ap.offset, ap=dims)


def _patch_act_tables():
    """Order table sets so natural_log_exp_and_others (contains every
    function this kernel uses) is chosen for all activations — avoids
    mid-kernel ACT table reloads."""
    import concourse.hw_specs as hw_specs
    orig = hw_specs.get_activation_tables
    if getattr(bacc, "_act_tables_patched", False):
        return

    AF_ = mybir.ActivationFunctionType

    def filtered(arch):
        tabs = orig(arch)
        out = {}
        for k, v in tabs.items():
            if k == "exp_and_others":
                v = {f for f in v if f != AF_.Exp}
            elif k == "natural_log":
                v = {f for f in v if f != AF_.Ln}
            out[k] = v
        return out

    bacc.get_activation_tables = filtered
    bacc._act_tables_patched = True


def _build():
    KPHASE = int(os.environ.get("KPHASE", "5"))
    if os.environ.get('ACTPATCH', '1') == '1':
        _patch_act_tables()
    nc = bacc.Bacc(None, target_bir_lowering=False, debug=False)

    x_d = nc.dram_tensor("x", [BPC, T, D], f32, kind="ExternalInput")
    wqk_d = nc.dram_tensor("wqk", [P, 2, 4, NK, P // 2], mybir.dt.uint32, kind="ExternalInput")
    qkb_d = nc.dram_tensor("qkb", [P, 2, 4], f32, kind="ExternalInput")
    wvt_d = nc.dram_tensor("wvt", [P, NK, D // 2], mybir.dt.uint32, kind="ExternalInput")
    vb_d = nc.dram_tensor("vb", [D], f32, kind="ExternalInput")
    wpt_d = nc.dram_tensor("wpt", [P, 4, D], f32r, kind="ExternalInput")
    bproj_d = nc.dram_tensor("bproj", [D], f32, kind="ExternalInput")
    w1t_d = nc.dram_tensor("w1t", [P, NK, NM, P // 2], mybir.dt.uint32, kind="ExternalInput")
    b1c_d = nc.dram_tensor("b1c", [P, NM], f32, kind="ExternalInput")
    w2t_d = nc.dram_tensor("w2t", [P, NM, D // 2], mybir.dt.uint32, kind="ExternalInput")
    b2_d = nc.dram_tensor("b2", [D], f32, kind="ExternalInput")
    ind_d = nc.dram_tensor("ind", [H, 4, P], f32r, kind="ExternalInput")
    identb_d = nc.dram_tensor("identb", [P, P], bf16, kind="ExternalInput")
    maskb_d = nc.dram_tensor("maskb", [P, P], bf16, kind="ExternalInput")
    maskw_d = nc.dram_tensor("maskw", [P, 2 * P], bf16, kind="ExternalInput")
    ones_d = nc.dram_tensor("ones1", [H], bf16, kind="ExternalInput")
    zeros_d = nc.dram_tensor("zeros1", [T], bf16, kind="ExternalInput")
    out_d = nc.dram_tensor("out", [BPC, T, D], f32, kind="ExternalOutput")

    with tile.TileContext(nc) as tc:
        import contextlib
        ctx = contextlib.ExitStack()
        with ctx:
            const = ctx.enter_context(tc.tile_pool(name="const", bufs=1))
            pool1 = ctx.enter_context(tc.tile_pool(name="pool1", bufs=1))
            pool2 = ctx.enter_context(tc.tile_pool(name="pool2", bufs=2))
            pool3 = ctx.enter_context(tc.tile_pool(name="pool3", bufs=3))
            pool_r = ctx.enter_context(tc.tile_pool(name="pool_r", bufs=1))
            pool_e = ctx.enter_context(tc.tile_pool(name="pool_e", bufs=3))
            ps_sc = ctx.enter_context(tc.tile_pool(name="ps_sc", bufs=2, space="PSUM"))
            ps_av = ctx.enter_context(tc.tile_pool(name="ps_av", bufs=2, space="PSUM"))
            ps_ms = ctx.enter_context(tc.tile_pool(name="ps_ms", bufs=4, space="PSUM"))

            # ---- resident constants ----
            qkb_t = const.tile([P, 2, 4], f32)
            wvt_t = const.tile([P, NK, D], bf16)
            vb_t = const.tile([P, D], f32)
            wpt_t = const.tile([P, 4, D], f32r)
            bproj_t = const.tile([P, D], f32)
            b1c_t = const.tile([P, NM], f32)
            b2_t = const.tile([P, D], f32)
            ind_t = const.tile([H, 4, P], f32r)
            identb_t = const.tile([P, P], bf16)
            maskb_t = const.tile([P, P], bf16)
            maskw_t = const.tile([P, 2 * P], bf16)
            w1t_t = const.tile([P, NK, NM, P], bf16)
            w2t_t = const.tile([P, NM, D], bf16)
            eps_t = const.tile([P, 1], f32)
            nc.vector.memset(eps_t[:, :], EPS)
            negc_t = const.tile([P, 1], f32)
            nc.vector.memset(negc_t[:, :], -C)

            def layernorm_to_featT(src_t, tag_prefix):
                """src_t [128, NT, 448] fp32 -> featT [128, NK, 1024] f32r."""
                mv_t = pool3.tile([P, NT, 2], f32, tag="mv")
                for i in range(NT):
                    st = pool3.tile([P, 6], f32, tag="stats")
                    nc.vector.bn_stats(out=st[:, :], in_=src_t[:, i, :])
                    nc.vector.bn_aggr(out=mv_t[:, i, :], in_=st[:, :])
                lg_t = pool3.tile([P, NT], f32, tag="lg")
                nc.scalar.activation(lg_t[:, :], mv_t[:, :, 1], AF.Ln, bias=eps_t[:, 0:1])
                rstd_t = pool3.tile([P, NT], f32, tag="rstd")
                nc.scalar.activation(rstd_t[:, :], lg_t[:, :], AF.Exp, scale=-0.5)

                ft = pool1.tile([P, NK, T], bf16, tag="featT")
                nc.gpsimd.dma_start(
                    out=ft[64:128, 3, :],
                    in_=_pb(zeros_d[:], 64))
                for g in range(2):        # groups of 4 t-tiles
                    for kk in range(NK):
                        w = 128 if kk < 3 else 64
                        pt = ps_ms.tile([P, CH], bf16, tag="ps_ms")
                        for ii in range(4):
                            i = 4 * g + ii
                            h_t = pool3.tile([P, P], bf16, tag="h")
                            nc.vector.tensor_scalar(
                                out=h_t[:, 0:w],
                                in0=src_t[:, i, 128 * kk:128 * kk + w],
                                scalar1=mv_t[:, i, 0:1],
                                scalar2=rstd_t[:, i:i + 1],
                                op0=ALU.subtract, op1=ALU.mult)
                            nc.tensor.transpose(
                                pt[0:w, 128 * ii:128 * ii + 128],
                                h_t[:, 0:w], identb_t[:, :])
                        nc.vector.tensor_copy(
                            ft[0:w, kk, CH * g:CH * g + CH], pt[0:w, :])
                return ft

            for b in range(BPC):
                # ---- load x (also the residual base tile) ----
                xb_t = pool2.tile([P, NT, D], f32, tag="resid")
                nc.sync.dma_start(
                    out=xb_t[:, :, :],
                    in_=x_d[b].rearrange("(n p) d -> p n d", p=P))

                if b == 0:
                    nc.sync.dma_start(out=identb_t[:, :], in_=identb_d[:, :])
                    nc.sync.dma_start(out=qkb_t[:, :, :], in_=qkb_d[:, :, :])
                    nc.sync.dma_start(out=maskb_t[:, :], in_=maskb_d[:, :])
                    nc.sync.dma_start(out=maskw_t[:, :], in_=maskw_d[:, :])
                    nc.sync.dma_start(out=ind_t[:, :, :], in_=ind_d[:, :, :])
                    nc.sync.dma_start(out=b1c_t[:, :], in_=b1c_d[:, :])
                    nc.sync.dma_start(
                        out=wvt_t[:, :, :].bitcast(mybir.dt.uint32),
                        in_=wvt_d[:, :, :])
                    nc.sync.dma_start(out=wpt_t[:, :, :], in_=wpt_d[:, :, :])
                    nc.gpsimd.dma_start(out=vb_t[:, :], in_=_pb(vb_d[:], P))
                    nc.gpsimd.dma_start(out=bproj_t[:, :], in_=_pb(bproj_d[:], P))
                    nc.gpsimd.dma_start(out=b2_t[:, :], in_=_pb(b2_d[:], P))

                # ---- LN1 + transpose ----
                hT_t = layernorm_to_featT(xb_t, "ln1")

                # xb := x + b_proj (in place; after LN1 reads)
                for i in range(NT):
                    nc.vector.tensor_add(xb_t[:, i, :], xb_t[:, i, :], bproj_t[:, :])

                if KPHASE < 2:
                    continue
                # ---- QKV ----
                wqk_t = pool1.tile([P, 2, 4, NK, P], bf16, tag="shA")
                for qk in range(2):
                    for prx in range(4):
                        nc.sync.dma_start(
                            out=wqk_t[:, qk, prx, :, :].bitcast(mybir.dt.uint32),
                            in_=wqk_d[:, qk, prx, :, :])
                qT_t = pool1.tile([P, 4, T], bf16, tag="shQ")
                kT_t = pool1.tile([P, 4, T], bf16, tag="shK")
                for pr in range(4):
                    for c in range(2):
                        pq = ps_ms.tile([P, CH], f32, tag="ps_ms")
                        for kk in range(NK):
                            nc.tensor.matmul(
                                pq[:, :], wqk_t[:, 0, pr, kk, :],
                                hT_t[:, kk, CH * c:CH * c + CH],
                                start=(kk == 0), stop=(kk == NK - 1))
                        nc.scalar.activation(
                            qT_t[:, pr, CH * c:CH * c + CH], pq[:, :],
                            AF.Identity, bias=qkb_t[:, 0, pr:pr + 1])
                        pk = ps_ms.tile([P, CH], f32, tag="ps_ms")
                        for kk in range(NK):
                            nc.tensor.matmul(
                                pk[:, :], wqk_t[:, 1, pr, kk, :],
                                hT_t[:, kk, CH * c:CH * c + CH],
                                start=(kk == 0), stop=(kk == NK - 1))
                        nc.vector.tensor_scalar(
                            out=kT_t[:, pr, CH * c:CH * c + CH], in0=pk[:, :],
                            scalar1=qkb_t[:, 1, pr:pr + 1], scalar2=None,
                            op0=ALU.add)

                if b == 0:
                    for kk in range(NK):
                        nc.sync.dma_start(
                            out=w1t_t[:, kk, :, :].bitcast(mybir.dt.uint32),
                            in_=w1t_d[:, kk, :, :])
                    for g in range(2):
                        nc.sync.dma_start(
                            out=w2t_t[:, 7 * g:7 * g + 7, :].bitcast(mybir.dt.uint32),
                            in_=w2t_d[:, 7 * g:7 * g + 7, :])

                # ---- V (token-major, with ones column) ----
                vext_t = pool1.tile([P, NT, H, 57], bf16, tag="vext")
                nc.gpsimd.dma_start(
                    out=vext_t[:, :, :, 56:57],
                    in_=bass.AP(tensor=ones_d[:].tensor, offset=0,
                                ap=[[0, P], [0, NT * H], [1, 1]]))
                for i in range(NT):
                    pv = ps_ms.tile([P, CH], f32, tag="ps_ms")
                    for kk in range(NK):
                        nc.tensor.matmul(
                            pv[:, 0:D], hT_t[:, kk, 128 * i:128 * i + 128],
                            wvt_t[:, kk, :],
                            start=(kk == 0), stop=(kk == NK - 1))
                    nc.vector.tensor_add(
                        vext_t[:, i, :, 0:56],
                        pv[:, 0:D].rearrange("p (h e) -> p h e", h=H),
                        vb_t[:, :].rearrange("p (h e) -> p h e", h=H))

                if KPHASE < 3:
                    continue
                # ---- attention ----
                attnT_t = pool1.tile([P, 4, T], f32r, tag="shA")
                l_t = pool1.tile([H, T], f32r, tag="l")
                for pr in range(4):
                    for c in range(2):
                        pav = ps_av.tile([P, CH], f32, tag="ps_av")
                        for half in range(2):
                            h = 2 * pr + half
                            poff = 64 * half
                            jmax = 4 * c + 3
                            e_t = pool_e.tile([P, 8, CH], bf16, tag="shE")
                            for j in range(jmax + 1):
                                co = max(0, 128 * (j - 4 * c))
                                if co == 384:
                                    co = 256   # keep matmul N >= 256 for f32r speed
                                pss = ps_sc.tile([P, CH], f32, tag="ps_sc")
                                diag = j >= 4 * c
                                nc.tensor.matmul(
                                    pss[:, co:CH],
                                    kT_t[poff:poff + 56, pr, 128 * j:128 * j + 128],
                                    qT_t[poff:poff + 56, pr, CH * c + co:CH * c + CH],
                                    start=True, stop=not diag)
                                if diag:
                                    dco = 128 * (j - 4 * c)
                                    if dco == 384:
                                        nc.tensor.matmul(
                                            pss[:, 256:512],
                                            identb_t[:, :], maskw_t[:, :],
                                            start=False, stop=True)
                                    else:
                                        nc.tensor.matmul(
                                            pss[:, dco:dco + 128],
                                            identb_t[:, :], maskb_t[:, :],
                                            start=False, stop=True)
                                nc.scalar.activation(
                                    e_t[:, j, co:CH], pss[:, co:CH],
                                    AF.Exp, scale=SCALE, bias=negc_t[:, 0:1])
                            for j in range(jmax + 1):
                                co = max(0, 128 * (j - 4 * c))
                                if co == 384:
                                    co = 256
                                nc.tensor.matmul(
                                    pav[poff:poff + 57, co:CH],
                                    vext_t[:, j, h, :], e_t[:, j, co:CH],
                                    tile_position=(0, poff),
                                    start=(j == 0), stop=(j == jmax))
                        nc.scalar.copy(
                            attnT_t[0:57, pr, CH * c:CH * c + CH], pav[0:57, :])
                        nc.vector.tensor_copy(
                            attnT_t[64:121, pr, CH * c:CH * c + CH], pav[64:121, :])
                        for half in range(2):
                            nc.gpsimd.dma_start(
                                out=l_t[2 * pr + half:2 * pr + half + 1,
                                        CH * c:CH * c + CH],
                                in_=attnT_t[56 + 64 * half:57 + 64 * half, pr,
                                            CH * c:CH * c + CH])

                # recip of softmax denominators: 1/l = exp(-log(l))
                nc.scalar.activation(l_t[:, :], l_t[:, :], AF.Ln)
                nc.scalar.activation(l_t[:, :], l_t[:, :], AF.Exp, scale=-1.0)

                for pr in range(4):
                    for c in range(2):
                        pb = ps_ms.tile([P, CH], f32, tag="ps_ms")
                        nc.tensor.matmul(
                            pb[:, :], ind_t[:, pr, :],
                            l_t[:, CH * c:CH * c + CH],
                            start=True, stop=True)
                        nc.vector.tensor_mul(
                            attnT_t[0:57, pr, CH * c:CH * c + CH],
                            attnT_t[0:57, pr, CH * c:CH * c + CH], pb[0:57, :])
                        nc.vector.tensor_mul(
                            attnT_t[64:121, pr, CH * c:CH * c + CH],
                            attnT_t[64:121, pr, CH * c:CH * c + CH], pb[64:121, :])

                if KPHASE < 4:
                    continue
                # ---- proj + residual ----
                x2_t = pool2.tile([P, NT, D], f32, tag="resid")
                for i in range(NT):
                    ppe = ps_ms.tile([P, CH], f32, tag="ps_ms")
                    ppo = ps_ms.tile([P, CH], f32, tag="ps_ms")
                    for half in range(2):
                        poff = 64 * half
                        pp = ppe if half == 0 else ppo
                        for pr in range(4):
                            nc.tensor.matmul(
                                pp[:, 0:D],
                                attnT_t[poff:poff + 56, pr, 128 * i:128 * i + 128],
                                wpt_t[poff:poff + 56, pr, :],
                                start=(pr == 0), stop=(pr == 3))
                    t1_t = pool2.tile([P, D], f32, tag="projtmp")
                    nc.vector.tensor_add(t1_t[:, :], ppe[:, 0:D], xb_t[:, i, :])
                    nc.vector.tensor_add(x2_t[:, i, :], ppo[:, 0:D], t1_t[:, :])

                # ---- LN2 + transpose ----
                h2T_t = layernorm_to_featT(x2_t, "ln2")

                # x2 := x2 + b2 (in place; after LN2 reads)
                for i in range(NT):
                    nc.vector.tensor_add(x2_t[:, i, :], x2_t[:, i, :], b2_t[:, :])

                if KPHASE < 5:
                    continue
                # ---- MLP ----
                for c2 in range(2):          # 512-wide t-chunks
                    rel_t = pool_r.tile([P, NM, CH], bf16, tag="reluT")
                    for m in range(NM):
                        pf = ps_ms.tile([P, CH], f32, tag="ps_ms")
                        for kk in range(NK):
                            nc.tensor.matmul(
                                pf[:, :], w1t_t[:, kk, m, :],
                                h2T_t[:, kk, CH * c2:CH * c2 + CH],
                                start=(kk == 0), stop=(kk == NK - 1))
                        if m % 2 == 0:
                            nc.scalar.activation(
                                rel_t[:, m, :], pf[:, :],
                                AF.Relu, bias=b1c_t[:, m:m + 1])
                        else:
                            nc.vector.tensor_scalar(
                                out=rel_t[:, m, :], in0=pf[:, :],
                                scalar1=b1c_t[:, m:m + 1], scalar2=0.0,
                                op0=ALU.add, op1=ALU.max)
                    for ii in range(4):
                        i = 4 * c2 + ii
                        po = ps_ms.tile([P, CH], f32, tag="ps_ms")
                        for k14 in range(NM):
                            nc.tensor.matmul(
                                po[:, 0:D],
                                rel_t[:, k14, 128 * ii:128 * ii + 128],
                                w2t_t[:, k14, :],
                                start=(k14 == 0), stop=(k14 == NM - 1))
                        o_t = pool2.tile([P, D], f32, tag="outt")
                        nc.vector.tensor_add(o_t[:, :], po[:, 0:D], x2_t[:, i, :])
                        nc.gpsimd.dma_start(
                            out=out_d[b, 128 * i:128 * i + 128, :], in_=o_t[:, :])

    nc.finalize()
    return nc


_CACHE = {}


def run(inputs, trace=False):
    if "nc" not in _CACHE:
        _CACHE["nc"] = _build()
    nc = _CACHE["nc"]
    host = _prep({k: np.asarray(v) for k, v in inputs.items()})
    x = np.asarray(inputs["x"], np.float32)
    in_maps = []
    for cidx in range(NCORE):
        m = dict(host)
        m["x"] = np.ascontiguousarray(x[BPC * cidx:BPC * cidx + BPC])
        in_maps.append(m)
    r = run_bass_kernel_spmd(nc, in_maps, list(range(NCORE)), trace=trace)
    out = np.concatenate([r.results[cidx]["out"] for cidx in range(NCORE)], axis=0)
    return out, r


def kernel(**inputs):
    out, _ = run(inputs, trace=False)
    return out
